# revision 19
# baseline (speedup 1.0000x reference)
"""Clements-mesh kernel for Trainium2 (8 NeuronCores, data-parallel).

The reference applies 64 layers of 2x2 Givens-like rotations (alternating
even/odd pair offsets) to x [32768, 256].  Each layer is right-multiplication
by a 256x256 block-diagonal orthogonal matrix U_l, so the whole network is
out = x @ (U_0 @ U_1 @ ... @ U_63) = x @ M with M a dense 256x256 matrix that
only depends on the tiny theta [64, 128].  M is built on host in float64;
the device kernel is a single [4096, 256] @ [256, 256] matmul per core.

Precision: the correctness gate is rel_err < 2e-2, so both x and M are sent
as single bf16 (RTNE) and the result is rounded to bf16 before the output
DMA; accumulation is exact f32 in PSUM.  Measured end-to-end rel err vs the
reference is ~2.9e-3 (7x margin).  This halves HBM traffic vs an x-hi/lo
split with f32 output: 2.2 MiB in + 2.1 MiB out per core, ~12 us at the
~360 GB/s per-core DMA roofline, which is what the kernel is bound by.

Device layout: TensorE contracts over the partition dim of both operands, so
x is shipped feature-major (host pre-transpose) and column-packed in DMA
stream order so every input chunk is ONE contiguous DMA:
  xin [128, 8704] bf16 = [M_kc0 | M_kc1 | X0_kc0 | X0_kc1 | ... | X4_kc1]
where kc = contraction chunk of 128 features and Xi are batch-column chunks
of width CHUNK_W[i].  out^T[j, b] = sum_k M[k, j] x^T[k, b] accumulates over
kc0+kc1 into one PSUM bank per (512-batch block, output-feature half); banks
are drained (with f32->bf16 cast) to SBUF by DVE (jc0) / ACT (jc1) since DMA
cannot read PSUM, then DMAed out feature-major; the host transposes back and
upcasts to f32 while gathering.

Scheduling: hand-built engine programs with explicit semaphores, no Tile
barriers.  The all-engine init barrier + dma_reset of earlier versions
(~3.5 us) is replaced by a semaphore gate: GpSimd clears the data semaphores
then raises start_sem; everything except the first input DMA (receipted on
its own never-start-cleared c0_sem) is gated behind it.  End-of-run GpSimd
clears make the NEFF re-executable; a reference-free row-norm self-check
with retry in kernel() guards the rare stale-device-state corruption.
"""

import sys

import numpy as np

if "/opt/trn_rl_repo" not in sys.path:
    sys.path.insert(0, "/opt/trn_rl_repo")

import concourse.bass as bass
import concourse.mybir as mybir
from concourse.tile import TileContext

D = 256          # feature dim
B = 32768        # batch
NCORES = 8
BS = B // NCORES  # 4096 batch rows per core
P = 128          # SBUF partitions
NB = 512         # batch columns per matmul (one fp32 PSUM bank)
NBLK = BS // NB  # 8 batch blocks
F32 = mybir.dt.float32
BF16 = mybir.dt.bfloat16

# xin column layout: [M_kc0 | M_kc1 | b0_kc0 | b0_kc1 | b1_kc0 | b1_kc1 |
# ... | b7_kc1] — the two 256-col M blocks, then per 512-batch-block pairs
# of contraction halves, in exact PE consumption order.  DMA boundaries
# (below) are chosen so the PE's first matmul depends only on the first
# 256 KB, and later transfers stay ahead of PE consumption (a >100ns PE
# idle gap drops the p-state from 2.4 to 1.2 GHz for ~1-3 us).
XIN_W = 2 * D + 2 * BS  # 8704

# Input DMA column ranges: d0 = M + b0_kc0, d1 = b0_kc1, then one DMA per
# batch block b1..b7 so per-block receipts release the PE as early as
# possible.  d0/d1 are receipted on c0_sem (+16 each), the rest on in_sem
# (+16 each).
IN_DMAS = [
    (0, 1024),
    (1024, 1536),
    (1536, 2560),
    (2560, 3584),
    (3584, 4608),
    (4608, 5632),
    (5632, 6656),
    (6656, 7680),
    (7680, XIN_W),
]
# in_sem threshold (x16) the PE must reach before starting batch block bb
# (b0 is handled specially via c0_sem).
_BB_THR = [0, 1, 2, 3, 4, 5, 6, 7]


def _xcol(bb: int, kc: int) -> int:
    return 2 * D + bb * 2 * NB + kc * NB


_NC_CACHE = {}


def _fused_matrix(theta: np.ndarray) -> np.ndarray:
    """M = U_0 @ U_1 @ ... @ U_63 in float64."""
    theta = np.asarray(theta, dtype=np.float64)
    M = np.eye(D, dtype=np.float64)
    for layer in range(theta.shape[0]):
        th = theta[layer]
        if layer % 2 == 0:
            npairs = D // 2
            i_idx = np.arange(0, D - 1, 2)
        else:
            npairs = D // 2 - 1
            i_idx = np.arange(1, D - 2, 2)
        j_idx = i_idx + 1
        c = np.cos(2.0 * th[:npairs])
        s = np.sin(2.0 * th[:npairs])
        Mi = M[:, i_idx].copy()
        Mj = M[:, j_idx]
        M[:, i_idx] = c * Mi + s * Mj
        M[:, j_idx] = s * Mi - c * Mj
    return M


def _legalize_waits(nc: bass.Bass, max_waits: int = 1) -> None:
    """Split instructions carrying more than ``max_waits`` sync waits.

    This walrus build rejects instructions with multiple sync-wait commands.
    Excess waits move to injected same-engine NoOps immediately before the
    instruction, which is semantically identical: the engine blocks on each
    wait in sequence before executing the original instruction.
    """
    for fn in nc.m.functions:
        for blk in fn.blocks:
            insts = blk.instructions
            i = 0
            while i < len(insts):
                inst = insts[i]
                si = inst.sync_info
                if si is not None and len(si.on_wait) > max_waits:
                    waits = list(si.on_wait)
                    keep, extra = waits[-max_waits:], waits[:-max_waits]
                    for k, w in enumerate(extra):
                        nop = mybir.InstNoOp(
                            name=f"{inst.name}-waitsplit-{k}", ins=[], outs=[]
                        )
                        nop.engine = inst.engine
                        nop.sync_info = mybir.SyncInfo(on_wait=[w], on_update=[])
                        insts.insert(i, nop)
                        i += 1
                    inst.sync_info = mybir.SyncInfo(
                        on_wait=keep, on_update=list(si.on_update)
                    )
                i += 1


def _strip_barriers(nc: bass.Bass) -> None:
    """Remove ALL all-engine EVSEM barrier butterflies + drains.

    Ordering is carried entirely by our semaphore protocol: GpSimd's
    start-of-run semaphore clears gate every semaphore producer via
    start_sem (the one ungated input DMA receipts on c0_sem, which is
    never start-cleared), and GpSimd's end-of-run clears run after the
    final output-DMA write receipt.
    """
    for fn in nc.m.functions:
        for blk in fn.blocks:
            insts = blk.instructions
            keep = [
                i
                for i in insts
                if not (
                    type(i).__name__ == "InstDrain"
                    or (
                        type(i).__name__ == "InstEventSemaphore"
                        and i.name.startswith("barrier")
                    )
                )
            ]
            if len(keep) != len(insts):
                insts[:] = keep


def _build_nc_raw() -> bass.Bass:
    from contextlib import ExitStack

    nc = bass.Bass()
    xin = nc.declare_dram_parameter("xin", [P, XIN_W], BF16, isOutput=False)
    outT = nc.declare_dram_parameter("outT", [2, P, BS], BF16, isOutput=True)

    NWARM = 7       # full-size (512-row) p-state warmup matmuls
    NWARM_FINE = 6  # quarter-size tail warmups for a fine-grained hand-off
    # PSUM banks (per jc) per out-DMA; tapered so the final transfer (which
    # the kernel-end drain effectively waits behind) is a single 128 KB bank.
    OGS = [2, 2, 2, 2]
    assert sum(OGS) == NBLK

    with ExitStack() as ctx:
        x_sb = ctx.enter_context(nc.sbuf_tensor("x_sb", [P, XIN_W], BF16))
        o_sb = ctx.enter_context(nc.sbuf_tensor("o_sb", [P, 2 * BS], BF16))
        ps = [
            ctx.enter_context(nc.psum_tensor(f"ps{b}", [P, NB], F32))
            for b in range(8)
        ]
        c0_sem = ctx.enter_context(nc.semaphore("c0_sem"))
        in_sem = ctx.enter_context(nc.semaphore("in_sem"))
        pe_sem = ctx.enter_context(nc.semaphore("pe_sem"))
        dve_sem = ctx.enter_context(nc.semaphore("dve_sem"))
        act_sem = ctx.enter_context(nc.semaphore("act_sem"))
        out_sem = ctx.enter_context(nc.semaphore("out_sem"))
        start_sem = ctx.enter_context(nc.semaphore("start_sem"))
        block = ctx.enter_context(nc.Block())

        # Group g = 2*bb + jc fills PSUM bank g % 8 with kc0+kc1 accumulated
        # matmuls; jc0 banks drain on DVE, jc1 banks on ACT (f32 -> bf16).

        @block.sync
        def _(sp):
            # The first two DMAs (M + b0_kc0, then b0_kc1) go out
            # immediately, receipted on c0_sem which GpSimd never clears at
            # start-of-run, so the start_sem gate cannot erase their
            # receipts.
            for di, (lo, hi) in enumerate(IN_DMAS[:2]):
                sp.dma_start(out=x_sb[:, lo:hi], in_=xin[:, lo:hi]).then_inc(
                    c0_sem, 16
                )
            # Everything else waits for GpSimd's semaphore clears.
            sp.wait_ge(start_sem, 1)
            for lo, hi in IN_DMAS[2:]:
                sp.dma_start(out=x_sb[:, lo:hi], in_=xin[:, lo:hi]).then_inc(
                    in_sem, 16
                )
            # Output DMAs issued in drain-completion order behind the input
            # stream.  Receipts land on out_sem which nothing waits on
            # (walrus requires a completion semaphore): the SP queue itself
            # retires only after the last pseudo-DMA transfer, and the
            # runtime's end-of-execution teardown quiesces the DMA path
            # before results are read.
            #
            # Crucially, hold ALL output DMAs until the input stream is
            # receipted: out transfers ride different hardware rings and the
            # DMA engines round-robin across rings, so an early out-DMA
            # steals bandwidth from the not-yet-transferred input chunks,
            # starving the PE (and its drains, and thus the tail) for
            # longer than the out transfer gains.  Total transfer work is
            # fixed; inputs are always on the critical path.
            sp.wait_ge(in_sem, 16 * (len(IN_DMAS) - 2))
            done = 0
            for og in OGS:
                for jc in range(2):
                    sem = dve_sem if jc == 0 else act_sem
                    sp.wait_ge(sem, done + og)
                    lo, hi = done * NB, (done + og) * NB
                    sp.dma_start(
                        out=outT[jc][:, lo:hi],
                        in_=o_sb[:, jc * BS + lo : jc * BS + hi],
                    ).then_inc(out_sem, 16)
                done += og

        @block.tensor
        def _(pe):
            def mm(bb, jc, kc, start, stop, inc=False):
                m = pe.matmul(
                    ps[(2 * bb + jc) % 8][:],
                    lhsT=x_sb[:, kc * D + jc * P : kc * D + (jc + 1) * P],
                    rhs=x_sb[:, _xcol(bb, kc) : _xcol(bb, kc) + NB],
                    start=start,
                    stop=stop,
                    skip_group_check=True,
                )
                if inc:
                    m.then_inc(pe_sem, 1)

            # Warm the PE p-state on garbage SBUF while the first input DMA
            # lands; bank 7's real group later overwrites this via
            # start=True.  The clocks of the preamble/DMA path and of the
            # warmup matmuls co-vary run to run, so a fixed warmup count
            # tracks the data-arrival time well; the tail of the warmup run
            # uses quarter-size matmuls so the hand-off to the first real
            # matmul is fine-grained (a PE idle gap would reset the p-state
            # ramp and double early matmul latency).
            for _w in range(NWARM):
                pe.matmul(
                    ps[7][:],
                    lhsT=x_sb[:, 0:P],
                    rhs=x_sb[:, 2 * D : 2 * D + NB],
                    start=True,
                    stop=True,
                )
            for _w in range(NWARM_FINE):
                pe.matmul(
                    ps[7][:, 0 : NB // 4],
                    lhsT=x_sb[:, 0:P],
                    rhs=x_sb[:, 2 * D : 2 * D + NB // 4],
                    start=True,
                    stop=True,
                )
            # Never produce a pe_sem increment before GpSimd's clears are
            # done (the c0 DMAs alone could otherwise race them).
            pe.wait_ge(start_sem, 1)
            # Block 0 runs in kc-pair order — both jc matmuls of kc0 first —
            # so work can start before b0_kc1 (second DMA) has landed.
            pe.wait_ge(c0_sem, 16)  # M blocks + b0_kc0
            mm(0, 0, 0, start=True, stop=False)
            mm(0, 1, 0, start=True, stop=False)
            pe.wait_ge(c0_sem, 32)  # b0_kc1
            mm(0, 0, 1, start=False, stop=True, inc=True)
            mm(0, 1, 1, start=False, stop=True, inc=True)
            last_thr = 0
            for bb in range(1, NBLK):
                if _BB_THR[bb] > last_thr:
                    last_thr = _BB_THR[bb]
                    pe.wait_ge(in_sem, 16 * last_thr)
                for jc in range(2):
                    g = 2 * bb + jc
                    if g >= 8:
                        prev = g - 8
                        sem = dve_sem if prev % 2 == 0 else act_sem
                        pe.wait_ge(sem, prev // 2 + 1)
                    mm(bb, jc, 0, start=True, stop=False)
                    mm(bb, jc, 1, start=False, stop=True, inc=True)

        @block.vector
        def _(dve):
            # Tiny delay op: give GpSimd's start-of-run clears time to land
            # before our first wait could observe stale values.
            dve.memset(o_sb[:, 0:8], 0.0)
            for i in range(NBLK):  # jc0 groups: g = 2i
                dve.wait_ge(pe_sem, 2 * i + 1)
                dve.tensor_copy(
                    o_sb[:, i * NB : (i + 1) * NB], ps[(2 * i) % 8][:]
                ).then_inc(dve_sem, 1)

        @block.scalar
        def _(act):
            # Tiny delay op; also triggers the one-time ACT table load well
            # before the first real drain needs it.
            act.copy(o_sb[:, BS : BS + 8], o_sb[:, BS : BS + 8])
            for i in range(NBLK):  # jc1 groups: g = 2i + 1
                act.wait_ge(pe_sem, 2 * i + 2)
                act.copy(
                    o_sb[:, BS + i * NB : BS + (i + 1) * NB], ps[(2 * i + 1) % 8][:]
                ).then_inc(act_sem, 1)

        @block.gpsimd
        def _(gp):
            # Start-of-run: zero the data semaphores, then release everything
            # via start_sem.  (c0_sem deliberately not cleared here: its DMA
            # is dispatched ungated, so a start-clear could erase in-flight
            # receipts.)
            for s in (in_sem, pe_sem, dve_sem, act_sem, out_sem):
                gp.sem_clear(s)
            gp.sem_inc(start_sem, 1)
            # End-of-run: once the last drains are done (i.e. every sem this
            # NEFF waits on has passed its final wait), reset the two
            # semaphores that are NOT start-of-run-cleared so the NEFF is
            # re-executable.  Finishes under the shadow of the final output
            # DMAs still retiring on the SP queue.
            gp.wait_ge(dve_sem, NBLK)
            gp.wait_ge(act_sem, NBLK)
            gp.sem_clear(c0_sem)
            gp.sem_clear(start_sem)

    _strip_barriers(nc)
    _legalize_waits(nc)
    return nc


def _get_nc() -> bass.Bass:
    if "nc" not in _NC_CACHE:
        _NC_CACHE["nc"] = _build_nc_raw()
    return _NC_CACHE["nc"]


def _make_in_maps(x: np.ndarray, theta: np.ndarray):
    import ml_dtypes

    x = np.ascontiguousarray(np.asarray(x), dtype=np.float32)
    mh = _fused_matrix(theta).astype(np.float32).astype(ml_dtypes.bfloat16)

    xr = x.reshape(NCORES, BS, D)
    in_maps = []
    for c in range(NCORES):
        xt = np.ascontiguousarray(xr[c].T).astype(ml_dtypes.bfloat16)
        cols = [mh[:P], mh[P:]]
        for bb in range(NBLK):
            cols.append(xt[:P, bb * NB : (bb + 1) * NB])
            cols.append(xt[P:, bb * NB : (bb + 1) * NB])
        in_maps.append({"xin": np.ascontiguousarray(np.concatenate(cols, axis=1))})
    return in_maps


def _gather(results) -> np.ndarray:
    out = np.empty((B, D), dtype=np.float32)
    for c in range(NCORES):
        oT = np.asarray(results[c]["outT"])  # [2, 128, 4096] bf16
        out[c * BS : (c + 1) * BS, :P] = oT[0].T.astype(np.float32)
        out[c * BS : (c + 1) * BS, P:] = oT[1].T.astype(np.float32)
    return out


def run(x: np.ndarray, theta: np.ndarray, trace: bool = False):
    """Returns (out, BassKernelResults)."""
    from concourse.bass_utils import run_bass_kernel_spmd

    in_maps = _make_in_maps(x, theta)
    res = run_bass_kernel_spmd(
        _get_nc(), in_maps, list(range(NCORES)), trace=trace
    )
    return _gather(res.results), res


def _self_check(x: np.ndarray, out: np.ndarray) -> bool:
    """M is a product of orthogonal factors, so ||out_row|| == ||x_row||.

    A cheap reference-free integrity check that catches the rare transient
    corruption seen when an execution races stale device state.  The bf16
    pipeline keeps the max row-norm deviation ~1.1e-3; real corruption is
    orders of magnitude larger.
    """
    xn = np.linalg.norm(np.asarray(x, dtype=np.float64), axis=1)
    on = np.linalg.norm(out.astype(np.float64), axis=1)
    return bool(np.max(np.abs(on - xn) / np.maximum(xn, 1e-6)) < 5e-3)


def kernel(x: np.ndarray, theta: np.ndarray) -> np.ndarray:
    for attempt in range(3):
        out, _ = run(x, theta, trace=False)
        if _self_check(x, out):
            return out
    return out


# revision 20
# speedup vs baseline: 1.0657x; 1.0657x over previous
"""Clements-mesh kernel for Trainium2 (8 NeuronCores, data-parallel).

The reference applies 64 layers of 2x2 Givens-like rotations (alternating
even/odd pair offsets) to x [32768, 256].  Each layer is right-multiplication
by a 256x256 block-diagonal orthogonal matrix U_l, so the whole network is
out = x @ (U_0 @ U_1 @ ... @ U_63) = x @ M with M a dense 256x256 matrix that
only depends on the tiny theta [64, 128].  M is built on host in float64;
the device kernel is a single [4096, 256] @ [256, 256] matmul per core.

Precision: the correctness gate is rel_err < 2e-2, so both x and M are sent
as single bf16 (RTNE) and the result is rounded to bf16 before the output
DMA; accumulation is exact f32 in PSUM.  Measured end-to-end rel err vs the
reference is ~2.9e-3 (7x margin).  This halves HBM traffic vs an x-hi/lo
split with f32 output: 2.2 MiB in + 2.1 MiB out per core, ~12 us at the
~360 GB/s per-core DMA roofline, which is what the kernel is bound by.

Device layout: TensorE contracts over the partition dim of both operands, so
x is shipped feature-major (host pre-transpose) and column-packed in DMA
stream order so every input chunk is ONE contiguous DMA:
  xin [128, 8704] bf16 = [M_kc0 | M_kc1 | X0_kc0 | X0_kc1 | ... | X4_kc1]
where kc = contraction chunk of 128 features and Xi are batch-column chunks
of width CHUNK_W[i].  out^T[j, b] = sum_k M[k, j] x^T[k, b] accumulates over
kc0+kc1 into one PSUM bank per (512-batch block, output-feature half); banks
are drained (with f32->bf16 cast) to SBUF by DVE (jc0) / ACT (jc1) since DMA
cannot read PSUM, then DMAed out feature-major; the host transposes back and
upcasts to f32 while gathering.

Scheduling: hand-built engine programs with explicit semaphores, no Tile
barriers.  The all-engine init barrier + dma_reset of earlier versions
(~3.5 us) is replaced by a semaphore gate: GpSimd clears the data semaphores
then raises start_sem; everything except the first input DMA (receipted on
its own never-start-cleared c0_sem) is gated behind it.  End-of-run GpSimd
clears make the NEFF re-executable; a reference-free row-norm self-check
with retry in kernel() guards the rare stale-device-state corruption.
"""

import sys

import numpy as np

if "/opt/trn_rl_repo" not in sys.path:
    sys.path.insert(0, "/opt/trn_rl_repo")

import concourse.bass as bass
import concourse.mybir as mybir
from concourse.tile import TileContext

D = 256          # feature dim
B = 32768        # batch
NCORES = 8
BS = B // NCORES  # 4096 batch rows per core
P = 128          # SBUF partitions
NB = 512         # batch columns per matmul (one fp32 PSUM bank)
NBLK = BS // NB  # 8 batch blocks
F32 = mybir.dt.float32
BF16 = mybir.dt.bfloat16

# xin column layout: [M_kc0 | M_kc1 | b0_kc0 | b0_kc1 | b1_kc0 | b1_kc1 |
# ... | b7_kc1] — the two 256-col M blocks, then per 512-batch-block pairs
# of contraction halves, in exact PE consumption order.  DMA boundaries
# (below) are chosen so the PE's first matmul depends only on the first
# 256 KB, and later transfers stay ahead of PE consumption (a >100ns PE
# idle gap drops the p-state from 2.4 to 1.2 GHz for ~1-3 us).
XIN_W = 2 * D + 2 * BS  # 8704

# Input DMA column ranges: d0 = M + b0_kc0, d1 = b0_kc1, then one DMA per
# batch block b1..b7 so per-block receipts release the PE as early as
# possible.  d0/d1 are receipted on c0_sem (+16 each), the rest on in_sem
# (+16 each).
IN_DMAS = [
    (0, 1024),
    (1024, 1536),
    (1536, 2560),
    (2560, 3584),
    (3584, 4608),
    (4608, 5632),
    (5632, 6656),
    (6656, 7680),
    (7680, XIN_W),
]
# in_sem threshold (x16) the PE must reach before starting batch block bb
# (b0 is handled specially via c0_sem).
_BB_THR = [0, 1, 2, 3, 4, 5, 6, 7]


def _xcol(bb: int, kc: int) -> int:
    return 2 * D + bb * 2 * NB + kc * NB


_NC_CACHE = {}


def _fused_matrix(theta: np.ndarray) -> np.ndarray:
    """M = U_0 @ U_1 @ ... @ U_63 in float64."""
    theta = np.asarray(theta, dtype=np.float64)
    M = np.eye(D, dtype=np.float64)
    for layer in range(theta.shape[0]):
        th = theta[layer]
        if layer % 2 == 0:
            npairs = D // 2
            i_idx = np.arange(0, D - 1, 2)
        else:
            npairs = D // 2 - 1
            i_idx = np.arange(1, D - 2, 2)
        j_idx = i_idx + 1
        c = np.cos(2.0 * th[:npairs])
        s = np.sin(2.0 * th[:npairs])
        Mi = M[:, i_idx].copy()
        Mj = M[:, j_idx]
        M[:, i_idx] = c * Mi + s * Mj
        M[:, j_idx] = s * Mi - c * Mj
    return M


def _legalize_waits(nc: bass.Bass, max_waits: int = 1) -> None:
    """Split instructions carrying more than ``max_waits`` sync waits.

    This walrus build rejects instructions with multiple sync-wait commands.
    Excess waits move to injected same-engine NoOps immediately before the
    instruction, which is semantically identical: the engine blocks on each
    wait in sequence before executing the original instruction.
    """
    for fn in nc.m.functions:
        for blk in fn.blocks:
            insts = blk.instructions
            i = 0
            while i < len(insts):
                inst = insts[i]
                si = inst.sync_info
                if si is not None and len(si.on_wait) > max_waits:
                    waits = list(si.on_wait)
                    keep, extra = waits[-max_waits:], waits[:-max_waits]
                    for k, w in enumerate(extra):
                        nop = mybir.InstNoOp(
                            name=f"{inst.name}-waitsplit-{k}", ins=[], outs=[]
                        )
                        nop.engine = inst.engine
                        nop.sync_info = mybir.SyncInfo(on_wait=[w], on_update=[])
                        insts.insert(i, nop)
                        i += 1
                    inst.sync_info = mybir.SyncInfo(
                        on_wait=keep, on_update=list(si.on_update)
                    )
                i += 1


def _strip_barriers(nc: bass.Bass) -> None:
    """Remove ALL all-engine EVSEM barrier butterflies + drains.

    Ordering is carried entirely by our semaphore protocol: GpSimd's
    start-of-run semaphore clears gate every semaphore producer via
    start_sem (the one ungated input DMA receipts on c0_sem, which is
    never start-cleared), and GpSimd's end-of-run clears run after the
    final output-DMA write receipt.
    """
    for fn in nc.m.functions:
        for blk in fn.blocks:
            insts = blk.instructions
            keep = [
                i
                for i in insts
                if not (
                    type(i).__name__ == "InstDrain"
                    or (
                        type(i).__name__ == "InstEventSemaphore"
                        and i.name.startswith("barrier")
                    )
                )
            ]
            if len(keep) != len(insts):
                insts[:] = keep


def _build_nc_raw() -> bass.Bass:
    from contextlib import ExitStack

    nc = bass.Bass()
    xin = nc.declare_dram_parameter("xin", [P, XIN_W], BF16, isOutput=False)
    outT = nc.declare_dram_parameter("outT", [2, P, BS], BF16, isOutput=True)

    NWARM = 7       # full-size (512-row) p-state warmup matmuls
    NWARM_FINE = 6  # quarter-size tail warmups for a fine-grained hand-off
    # PSUM banks (per jc) per out-DMA; tapered so the final transfer (which
    # the kernel-end drain effectively waits behind) is a single 128 KB bank.
    OGS = [2, 2, 2, 2]
    assert sum(OGS) == NBLK

    with ExitStack() as ctx:
        x_sb = ctx.enter_context(nc.sbuf_tensor("x_sb", [P, XIN_W], BF16))
        o_sb = ctx.enter_context(nc.sbuf_tensor("o_sb", [P, 2 * BS], BF16))
        ps = [
            ctx.enter_context(nc.psum_tensor(f"ps{b}", [P, NB], F32))
            for b in range(8)
        ]
        c0_sem = ctx.enter_context(nc.semaphore("c0_sem"))
        in_sem = ctx.enter_context(nc.semaphore("in_sem"))
        pe_sem = ctx.enter_context(nc.semaphore("pe_sem"))
        dve_sem = ctx.enter_context(nc.semaphore("dve_sem"))
        act_sem = ctx.enter_context(nc.semaphore("act_sem"))
        out_sem = ctx.enter_context(nc.semaphore("out_sem"))
        start_sem = ctx.enter_context(nc.semaphore("start_sem"))
        block = ctx.enter_context(nc.Block())

        # Group g = 2*bb + jc fills PSUM bank g % 8 with kc0+kc1 accumulated
        # matmuls; jc0 banks drain on DVE, jc1 banks on ACT (f32 -> bf16).

        @block.sync
        def _(sp):
            # The first two DMAs (M + b0_kc0, then b0_kc1) go out
            # immediately, receipted on c0_sem which GpSimd never clears at
            # start-of-run, so the start_sem gate cannot erase their
            # receipts.
            for di, (lo, hi) in enumerate(IN_DMAS[:2]):
                sp.dma_start(out=x_sb[:, lo:hi], in_=xin[:, lo:hi]).then_inc(
                    c0_sem, 16
                )
            # Everything else waits for GpSimd's semaphore clears.
            sp.wait_ge(start_sem, 1)
            for lo, hi in IN_DMAS[2:]:
                sp.dma_start(out=x_sb[:, lo:hi], in_=xin[:, lo:hi]).then_inc(
                    in_sem, 16
                )
            # Output DMAs issued in drain-completion order behind the input
            # stream.  Receipts land on out_sem which nothing waits on
            # (walrus requires a completion semaphore): the SP queue itself
            # retires only after the last pseudo-DMA transfer, and the
            # runtime's end-of-execution teardown quiesces the DMA path
            # before results are read.
            #
            # Crucially, hold output DMAs until the input stream is nearly
            # done: out transfers ride different hardware rings and the DMA
            # engines round-robin across rings, so an early out-DMA steals
            # bandwidth from not-yet-transferred input chunks, starving the
            # PE (and its drains, and thus the tail) for longer than the out
            # transfer gains.  Gating on the SECOND-TO-LAST input receipt
            # splices the first out transfer (issue + descriptor-gen latency
            # ~1.4 us) right behind the last input transfer, keeping the DMA
            # engines saturated to the end.
            sp.wait_ge(in_sem, 16 * (len(IN_DMAS) - 3))
            done = 0
            for og in OGS:
                for jc in range(2):
                    sem = dve_sem if jc == 0 else act_sem
                    sp.wait_ge(sem, done + og)
                    lo, hi = done * NB, (done + og) * NB
                    sp.dma_start(
                        out=outT[jc][:, lo:hi],
                        in_=o_sb[:, jc * BS + lo : jc * BS + hi],
                    ).then_inc(out_sem, 16)
                done += og

        @block.tensor
        def _(pe):
            def mm(bb, jc, kc, start, stop, inc=False):
                m = pe.matmul(
                    ps[(2 * bb + jc) % 8][:],
                    lhsT=x_sb[:, kc * D + jc * P : kc * D + (jc + 1) * P],
                    rhs=x_sb[:, _xcol(bb, kc) : _xcol(bb, kc) + NB],
                    start=start,
                    stop=stop,
                    skip_group_check=True,
                )
                if inc:
                    m.then_inc(pe_sem, 1)

            # Warm the PE p-state on garbage SBUF while the first input DMA
            # lands; bank 7's real group later overwrites this via
            # start=True.  The clocks of the preamble/DMA path and of the
            # warmup matmuls co-vary run to run, so a fixed warmup count
            # tracks the data-arrival time well; the tail of the warmup run
            # uses quarter-size matmuls so the hand-off to the first real
            # matmul is fine-grained (a PE idle gap would reset the p-state
            # ramp and double early matmul latency).
            for _w in range(NWARM):
                pe.matmul(
                    ps[7][:],
                    lhsT=x_sb[:, 0:P],
                    rhs=x_sb[:, 2 * D : 2 * D + NB],
                    start=True,
                    stop=True,
                )
            for _w in range(NWARM_FINE):
                pe.matmul(
                    ps[7][:, 0 : NB // 4],
                    lhsT=x_sb[:, 0:P],
                    rhs=x_sb[:, 2 * D : 2 * D + NB // 4],
                    start=True,
                    stop=True,
                )
            # Never produce a pe_sem increment before GpSimd's clears are
            # done (the c0 DMAs alone could otherwise race them).
            pe.wait_ge(start_sem, 1)
            # Block 0 runs in kc-pair order — both jc matmuls of kc0 first —
            # so work can start before b0_kc1 (second DMA) has landed.
            pe.wait_ge(c0_sem, 16)  # M blocks + b0_kc0
            mm(0, 0, 0, start=True, stop=False)
            mm(0, 1, 0, start=True, stop=False)
            pe.wait_ge(c0_sem, 32)  # b0_kc1
            mm(0, 0, 1, start=False, stop=True, inc=True)
            mm(0, 1, 1, start=False, stop=True, inc=True)
            last_thr = 0
            for bb in range(1, NBLK):
                if _BB_THR[bb] > last_thr:
                    last_thr = _BB_THR[bb]
                    pe.wait_ge(in_sem, 16 * last_thr)
                for jc in range(2):
                    g = 2 * bb + jc
                    if g >= 8:
                        prev = g - 8
                        sem = dve_sem if prev % 2 == 0 else act_sem
                        pe.wait_ge(sem, prev // 2 + 1)
                    mm(bb, jc, 0, start=True, stop=False)
                    mm(bb, jc, 1, start=False, stop=True, inc=True)

        @block.vector
        def _(dve):
            # Tiny delay op: give GpSimd's start-of-run clears time to land
            # before our first wait could observe stale values.
            dve.memset(o_sb[:, 0:8], 0.0)
            for i in range(NBLK):  # jc0 groups: g = 2i
                dve.wait_ge(pe_sem, 2 * i + 1)
                dve.tensor_copy(
                    o_sb[:, i * NB : (i + 1) * NB], ps[(2 * i) % 8][:]
                ).then_inc(dve_sem, 1)

        @block.scalar
        def _(act):
            # Tiny delay op; also triggers the one-time ACT table load well
            # before the first real drain needs it.
            act.copy(o_sb[:, BS : BS + 8], o_sb[:, BS : BS + 8])
            for i in range(NBLK):  # jc1 groups: g = 2i + 1
                act.wait_ge(pe_sem, 2 * i + 2)
                act.copy(
                    o_sb[:, BS + i * NB : BS + (i + 1) * NB], ps[(2 * i + 1) % 8][:]
                ).then_inc(act_sem, 1)

        @block.gpsimd
        def _(gp):
            # Start-of-run: zero the data semaphores, then release everything
            # via start_sem.  (c0_sem deliberately not cleared here: its DMA
            # is dispatched ungated, so a start-clear could erase in-flight
            # receipts.)
            for s in (in_sem, pe_sem, dve_sem, act_sem, out_sem):
                gp.sem_clear(s)
            gp.sem_inc(start_sem, 1)
            # End-of-run: once the last drains are done (i.e. every sem this
            # NEFF waits on has passed its final wait), reset the two
            # semaphores that are NOT start-of-run-cleared so the NEFF is
            # re-executable.  Finishes under the shadow of the final output
            # DMAs still retiring on the SP queue.
            gp.wait_ge(dve_sem, NBLK)
            gp.wait_ge(act_sem, NBLK)
            gp.sem_clear(c0_sem)
            gp.sem_clear(start_sem)

    _strip_barriers(nc)
    _legalize_waits(nc)
    return nc


def _get_nc() -> bass.Bass:
    if "nc" not in _NC_CACHE:
        _NC_CACHE["nc"] = _build_nc_raw()
    return _NC_CACHE["nc"]


def _make_in_maps(x: np.ndarray, theta: np.ndarray):
    import ml_dtypes

    x = np.ascontiguousarray(np.asarray(x), dtype=np.float32)
    mh = _fused_matrix(theta).astype(np.float32).astype(ml_dtypes.bfloat16)

    xr = x.reshape(NCORES, BS, D)
    in_maps = []
    for c in range(NCORES):
        xt = np.ascontiguousarray(xr[c].T).astype(ml_dtypes.bfloat16)
        cols = [mh[:P], mh[P:]]
        for bb in range(NBLK):
            cols.append(xt[:P, bb * NB : (bb + 1) * NB])
            cols.append(xt[P:, bb * NB : (bb + 1) * NB])
        in_maps.append({"xin": np.ascontiguousarray(np.concatenate(cols, axis=1))})
    return in_maps


def _gather(results) -> np.ndarray:
    out = np.empty((B, D), dtype=np.float32)
    for c in range(NCORES):
        oT = np.asarray(results[c]["outT"])  # [2, 128, 4096] bf16
        out[c * BS : (c + 1) * BS, :P] = oT[0].T.astype(np.float32)
        out[c * BS : (c + 1) * BS, P:] = oT[1].T.astype(np.float32)
    return out


def run(x: np.ndarray, theta: np.ndarray, trace: bool = False):
    """Returns (out, BassKernelResults)."""
    from concourse.bass_utils import run_bass_kernel_spmd

    in_maps = _make_in_maps(x, theta)
    res = run_bass_kernel_spmd(
        _get_nc(), in_maps, list(range(NCORES)), trace=trace
    )
    return _gather(res.results), res


def _self_check(x: np.ndarray, out: np.ndarray) -> bool:
    """M is a product of orthogonal factors, so ||out_row|| == ||x_row||.

    A cheap reference-free integrity check that catches the rare transient
    corruption seen when an execution races stale device state.  The bf16
    pipeline keeps the max row-norm deviation ~1.1e-3; real corruption is
    orders of magnitude larger.
    """
    xn = np.linalg.norm(np.asarray(x, dtype=np.float64), axis=1)
    on = np.linalg.norm(out.astype(np.float64), axis=1)
    return bool(np.max(np.abs(on - xn) / np.maximum(xn, 1e-6)) < 5e-3)


def kernel(x: np.ndarray, theta: np.ndarray) -> np.ndarray:
    for attempt in range(3):
        out, _ = run(x, theta, trace=False)
        if _self_check(x, out):
            return out
    return out


# revision 21
# speedup vs baseline: 1.0898x; 1.0226x over previous
"""Clements-mesh kernel for Trainium2 (8 NeuronCores, data-parallel).

The reference applies 64 layers of 2x2 Givens-like rotations (alternating
even/odd pair offsets) to x [32768, 256].  Each layer is right-multiplication
by a 256x256 block-diagonal orthogonal matrix U_l, so the whole network is
out = x @ (U_0 @ U_1 @ ... @ U_63) = x @ M with M a dense 256x256 matrix that
only depends on the tiny theta [64, 128].  M is built on host in float64;
the device kernel is a single [4096, 256] @ [256, 256] matmul per core.

Precision: the correctness gate is rel_err < 2e-2, so both x and M are sent
as single bf16 (RTNE) and the result is rounded to bf16 before the output
DMA; accumulation is exact f32 in PSUM.  Measured end-to-end rel err vs the
reference is ~2.9e-3 (7x margin).  This halves HBM traffic vs an x-hi/lo
split with f32 output: 2.2 MiB in + 2.1 MiB out per core, ~12 us at the
~360 GB/s per-core DMA roofline, which is what the kernel is bound by.

Device layout: TensorE contracts over the partition dim of both operands, so
x is shipped feature-major (host pre-transpose) and column-packed in DMA
stream order so every input chunk is ONE contiguous DMA:
  xin [128, 8704] bf16 = [M_kc0 | M_kc1 | X0_kc0 | X0_kc1 | ... | X4_kc1]
where kc = contraction chunk of 128 features and Xi are batch-column chunks
of width CHUNK_W[i].  out^T[j, b] = sum_k M[k, j] x^T[k, b] accumulates over
kc0+kc1 into one PSUM bank per (512-batch block, output-feature half); banks
are drained (with f32->bf16 cast) to SBUF by DVE (jc0) / ACT (jc1) since DMA
cannot read PSUM, then DMAed out feature-major; the host transposes back and
upcasts to f32 while gathering.

Scheduling: hand-built engine programs with explicit semaphores, no Tile
barriers.  The all-engine init barrier + dma_reset of earlier versions
(~3.5 us) is replaced by a semaphore gate: GpSimd clears the data semaphores
then raises start_sem; everything except the first input DMA (receipted on
its own never-start-cleared c0_sem) is gated behind it.  End-of-run GpSimd
clears make the NEFF re-executable; a reference-free row-norm self-check
with retry in kernel() guards the rare stale-device-state corruption.
"""

import sys

import numpy as np

if "/opt/trn_rl_repo" not in sys.path:
    sys.path.insert(0, "/opt/trn_rl_repo")

import concourse.bass as bass
import concourse.mybir as mybir
from concourse.tile import TileContext

D = 256          # feature dim
B = 32768        # batch
NCORES = 8
BS = B // NCORES  # 4096 batch rows per core
P = 128          # SBUF partitions
NB = 512         # batch columns per matmul (one fp32 PSUM bank)
NBLK = BS // NB  # 8 batch blocks
F32 = mybir.dt.float32
BF16 = mybir.dt.bfloat16

# xin column layout: [M_kc0 | M_kc1 | b0_kc0 | b0_kc1 | b1_kc0 | b1_kc1 |
# ... | b7_kc1] — the two 256-col M blocks, then per 512-batch-block pairs
# of contraction halves, in exact PE consumption order.  DMA boundaries
# (below) are chosen so the PE's first matmul depends only on the first
# 256 KB, and later transfers stay ahead of PE consumption (a >100ns PE
# idle gap drops the p-state from 2.4 to 1.2 GHz for ~1-3 us).
XIN_W = 2 * D + 2 * BS  # 8704

# Input DMA column ranges: d0 = M + b0_kc0, d1 = b0_kc1, then one DMA per
# batch block b1..b7 so per-block receipts release the PE as early as
# possible.  d0/d1 are receipted on c0_sem (+16 each), the rest on in_sem
# (+16 each).
IN_DMAS = [
    (0, 1024),
    (1024, 1536),
    (1536, 2560),
    (2560, 3584),
    (3584, 4608),
    (4608, 5632),
    (5632, 6656),
    (6656, 7680),
    (7680, XIN_W),
]
# in_sem threshold (x16) the PE must reach before starting batch block bb
# (b0 is handled specially via c0_sem).
_BB_THR = [0, 1, 2, 3, 4, 5, 6, 7]


def _xcol(bb: int, kc: int) -> int:
    return 2 * D + bb * 2 * NB + kc * NB


_NC_CACHE = {}


def _fused_matrix(theta: np.ndarray) -> np.ndarray:
    """M = U_0 @ U_1 @ ... @ U_63 in float64."""
    theta = np.asarray(theta, dtype=np.float64)
    M = np.eye(D, dtype=np.float64)
    for layer in range(theta.shape[0]):
        th = theta[layer]
        if layer % 2 == 0:
            npairs = D // 2
            i_idx = np.arange(0, D - 1, 2)
        else:
            npairs = D // 2 - 1
            i_idx = np.arange(1, D - 2, 2)
        j_idx = i_idx + 1
        c = np.cos(2.0 * th[:npairs])
        s = np.sin(2.0 * th[:npairs])
        Mi = M[:, i_idx].copy()
        Mj = M[:, j_idx]
        M[:, i_idx] = c * Mi + s * Mj
        M[:, j_idx] = s * Mi - c * Mj
    return M


def _legalize_waits(nc: bass.Bass, max_waits: int = 1) -> None:
    """Split instructions carrying more than ``max_waits`` sync waits.

    This walrus build rejects instructions with multiple sync-wait commands.
    Excess waits move to injected same-engine NoOps immediately before the
    instruction, which is semantically identical: the engine blocks on each
    wait in sequence before executing the original instruction.
    """
    for fn in nc.m.functions:
        for blk in fn.blocks:
            insts = blk.instructions
            i = 0
            while i < len(insts):
                inst = insts[i]
                si = inst.sync_info
                if si is not None and len(si.on_wait) > max_waits:
                    waits = list(si.on_wait)
                    keep, extra = waits[-max_waits:], waits[:-max_waits]
                    for k, w in enumerate(extra):
                        nop = mybir.InstNoOp(
                            name=f"{inst.name}-waitsplit-{k}", ins=[], outs=[]
                        )
                        nop.engine = inst.engine
                        nop.sync_info = mybir.SyncInfo(on_wait=[w], on_update=[])
                        insts.insert(i, nop)
                        i += 1
                    inst.sync_info = mybir.SyncInfo(
                        on_wait=keep, on_update=list(si.on_update)
                    )
                i += 1


def _strip_barriers(nc: bass.Bass) -> None:
    """Remove ALL all-engine EVSEM barrier butterflies + drains.

    Ordering is carried entirely by our semaphore protocol: GpSimd's
    start-of-run semaphore clears gate every semaphore producer via
    start_sem (the one ungated input DMA receipts on c0_sem, which is
    never start-cleared), and GpSimd's end-of-run clears run after the
    final output-DMA write receipt.
    """
    for fn in nc.m.functions:
        for blk in fn.blocks:
            insts = blk.instructions
            keep = [
                i
                for i in insts
                if not (
                    type(i).__name__ == "InstDrain"
                    or (
                        type(i).__name__ == "InstEventSemaphore"
                        and i.name.startswith("barrier")
                    )
                )
            ]
            if len(keep) != len(insts):
                insts[:] = keep


def _build_nc_raw() -> bass.Bass:
    from contextlib import ExitStack

    nc = bass.Bass()
    xin = nc.declare_dram_parameter("xin", [P, XIN_W], BF16, isOutput=False)
    outT = nc.declare_dram_parameter("outT", [2, P, BS], BF16, isOutput=True)

    NWARM = 7       # full-size (512-row) p-state warmup matmuls
    NWARM_FINE = 6  # quarter-size tail warmups for a fine-grained hand-off
    # PSUM banks (per jc) per out-DMA; tapered so the final transfer (which
    # the kernel-end drain effectively waits behind) is a single 128 KB bank.
    OGS = [2, 3, 3]
    assert sum(OGS) == NBLK

    with ExitStack() as ctx:
        x_sb = ctx.enter_context(nc.sbuf_tensor("x_sb", [P, XIN_W], BF16))
        o_sb = ctx.enter_context(nc.sbuf_tensor("o_sb", [P, 2 * BS], BF16))
        ps = [
            ctx.enter_context(nc.psum_tensor(f"ps{b}", [P, NB], F32))
            for b in range(8)
        ]
        c0_sem = ctx.enter_context(nc.semaphore("c0_sem"))
        in_sem = ctx.enter_context(nc.semaphore("in_sem"))
        pe_sem = ctx.enter_context(nc.semaphore("pe_sem"))
        dve_sem = ctx.enter_context(nc.semaphore("dve_sem"))
        act_sem = ctx.enter_context(nc.semaphore("act_sem"))
        out_sem = ctx.enter_context(nc.semaphore("out_sem"))
        start_sem = ctx.enter_context(nc.semaphore("start_sem"))
        block = ctx.enter_context(nc.Block())

        # Group g = 2*bb + jc fills PSUM bank g % 8 with kc0+kc1 accumulated
        # matmuls; jc0 banks drain on DVE, jc1 banks on ACT (f32 -> bf16).

        @block.sync
        def _(sp):
            # The first two DMAs (M + b0_kc0, then b0_kc1) go out
            # immediately, receipted on c0_sem which GpSimd never clears at
            # start-of-run, so the start_sem gate cannot erase their
            # receipts.
            for di, (lo, hi) in enumerate(IN_DMAS[:2]):
                sp.dma_start(out=x_sb[:, lo:hi], in_=xin[:, lo:hi]).then_inc(
                    c0_sem, 16
                )
            # Everything else waits for GpSimd's semaphore clears.
            sp.wait_ge(start_sem, 1)
            for lo, hi in IN_DMAS[2:]:
                sp.dma_start(out=x_sb[:, lo:hi], in_=xin[:, lo:hi]).then_inc(
                    in_sem, 16
                )
            # Output DMAs issued in drain-completion order behind the input
            # stream.  Receipts land on out_sem which nothing waits on
            # (walrus requires a completion semaphore): the SP queue itself
            # retires only after the last pseudo-DMA transfer, and the
            # runtime's end-of-execution teardown quiesces the DMA path
            # before results are read.
            #
            # Crucially, hold output DMAs until the input stream is nearly
            # done: out transfers ride different hardware rings and the DMA
            # engines round-robin across rings, so an early out-DMA steals
            # bandwidth from not-yet-transferred input chunks, starving the
            # PE (and its drains, and thus the tail) for longer than the out
            # transfer gains.  Gating on the SECOND-TO-LAST input receipt
            # splices the first out transfer (issue + descriptor-gen latency
            # ~1.4 us) right behind the last input transfer, keeping the DMA
            # engines saturated to the end.
            sp.wait_ge(in_sem, 16 * (len(IN_DMAS) - 3))
            done = 0
            for og in OGS:
                for jc in range(2):
                    sem = dve_sem if jc == 0 else act_sem
                    sp.wait_ge(sem, done + og)
                    lo, hi = done * NB, (done + og) * NB
                    sp.dma_start(
                        out=outT[jc][:, lo:hi],
                        in_=o_sb[:, jc * BS + lo : jc * BS + hi],
                    ).then_inc(out_sem, 16)
                done += og

        @block.tensor
        def _(pe):
            def mm(bb, jc, kc, start, stop, inc=False):
                m = pe.matmul(
                    ps[(2 * bb + jc) % 8][:],
                    lhsT=x_sb[:, kc * D + jc * P : kc * D + (jc + 1) * P],
                    rhs=x_sb[:, _xcol(bb, kc) : _xcol(bb, kc) + NB],
                    start=start,
                    stop=stop,
                    skip_group_check=True,
                )
                if inc:
                    m.then_inc(pe_sem, 1)

            # Warm the PE p-state on garbage SBUF while the first input DMA
            # lands; bank 7's real group later overwrites this via
            # start=True.  The clocks of the preamble/DMA path and of the
            # warmup matmuls co-vary run to run, so a fixed warmup count
            # tracks the data-arrival time well; the tail of the warmup run
            # uses quarter-size matmuls so the hand-off to the first real
            # matmul is fine-grained (a PE idle gap would reset the p-state
            # ramp and double early matmul latency).
            for _w in range(NWARM):
                pe.matmul(
                    ps[7][:],
                    lhsT=x_sb[:, 0:P],
                    rhs=x_sb[:, 2 * D : 2 * D + NB],
                    start=True,
                    stop=True,
                )
            for _w in range(NWARM_FINE):
                pe.matmul(
                    ps[7][:, 0 : NB // 4],
                    lhsT=x_sb[:, 0:P],
                    rhs=x_sb[:, 2 * D : 2 * D + NB // 4],
                    start=True,
                    stop=True,
                )
            # Never produce a pe_sem increment before GpSimd's clears are
            # done (the c0 DMAs alone could otherwise race them).
            pe.wait_ge(start_sem, 1)
            # Block 0 runs in kc-pair order — both jc matmuls of kc0 first —
            # so work can start before b0_kc1 (second DMA) has landed.
            pe.wait_ge(c0_sem, 16)  # M blocks + b0_kc0
            mm(0, 0, 0, start=True, stop=False)
            mm(0, 1, 0, start=True, stop=False)
            pe.wait_ge(c0_sem, 32)  # b0_kc1
            mm(0, 0, 1, start=False, stop=True, inc=True)
            mm(0, 1, 1, start=False, stop=True, inc=True)
            last_thr = 0
            for bb in range(1, NBLK):
                if _BB_THR[bb] > last_thr:
                    last_thr = _BB_THR[bb]
                    pe.wait_ge(in_sem, 16 * last_thr)
                for jc in range(2):
                    g = 2 * bb + jc
                    if g >= 8:
                        prev = g - 8
                        sem = dve_sem if prev % 2 == 0 else act_sem
                        pe.wait_ge(sem, prev // 2 + 1)
                    mm(bb, jc, 0, start=True, stop=False)
                    mm(bb, jc, 1, start=False, stop=True, inc=True)

        @block.vector
        def _(dve):
            # Tiny delay op: give GpSimd's start-of-run clears time to land
            # before our first wait could observe stale values.
            dve.memset(o_sb[:, 0:8], 0.0)
            for i in range(NBLK):  # jc0 groups: g = 2i
                dve.wait_ge(pe_sem, 2 * i + 1)
                dve.tensor_copy(
                    o_sb[:, i * NB : (i + 1) * NB], ps[(2 * i) % 8][:]
                ).then_inc(dve_sem, 1)

        @block.scalar
        def _(act):
            # Tiny delay op; also triggers the one-time ACT table load well
            # before the first real drain needs it.
            act.copy(o_sb[:, BS : BS + 8], o_sb[:, BS : BS + 8])
            for i in range(NBLK):  # jc1 groups: g = 2i + 1
                act.wait_ge(pe_sem, 2 * i + 2)
                act.copy(
                    o_sb[:, BS + i * NB : BS + (i + 1) * NB], ps[(2 * i + 1) % 8][:]
                ).then_inc(act_sem, 1)

        @block.gpsimd
        def _(gp):
            # Start-of-run: zero the data semaphores, then release everything
            # via start_sem.  (c0_sem deliberately not cleared here: its DMA
            # is dispatched ungated, so a start-clear could erase in-flight
            # receipts.)
            for s in (in_sem, pe_sem, dve_sem, act_sem, out_sem):
                gp.sem_clear(s)
            gp.sem_inc(start_sem, 1)
            # End-of-run: once the last drains are done (i.e. every sem this
            # NEFF waits on has passed its final wait), reset the two
            # semaphores that are NOT start-of-run-cleared so the NEFF is
            # re-executable.  Finishes under the shadow of the final output
            # DMAs still retiring on the SP queue.
            gp.wait_ge(dve_sem, NBLK)
            gp.wait_ge(act_sem, NBLK)
            gp.sem_clear(c0_sem)
            gp.sem_clear(start_sem)

    _strip_barriers(nc)
    _legalize_waits(nc)
    return nc


def _get_nc() -> bass.Bass:
    if "nc" not in _NC_CACHE:
        _NC_CACHE["nc"] = _build_nc_raw()
    return _NC_CACHE["nc"]


def _make_in_maps(x: np.ndarray, theta: np.ndarray):
    import ml_dtypes

    x = np.ascontiguousarray(np.asarray(x), dtype=np.float32)
    mh = _fused_matrix(theta).astype(np.float32).astype(ml_dtypes.bfloat16)

    xr = x.reshape(NCORES, BS, D)
    in_maps = []
    for c in range(NCORES):
        xt = np.ascontiguousarray(xr[c].T).astype(ml_dtypes.bfloat16)
        cols = [mh[:P], mh[P:]]
        for bb in range(NBLK):
            cols.append(xt[:P, bb * NB : (bb + 1) * NB])
            cols.append(xt[P:, bb * NB : (bb + 1) * NB])
        in_maps.append({"xin": np.ascontiguousarray(np.concatenate(cols, axis=1))})
    return in_maps


def _gather(results) -> np.ndarray:
    out = np.empty((B, D), dtype=np.float32)
    for c in range(NCORES):
        oT = np.asarray(results[c]["outT"])  # [2, 128, 4096] bf16
        out[c * BS : (c + 1) * BS, :P] = oT[0].T.astype(np.float32)
        out[c * BS : (c + 1) * BS, P:] = oT[1].T.astype(np.float32)
    return out


def run(x: np.ndarray, theta: np.ndarray, trace: bool = False):
    """Returns (out, BassKernelResults)."""
    from concourse.bass_utils import run_bass_kernel_spmd

    in_maps = _make_in_maps(x, theta)
    res = run_bass_kernel_spmd(
        _get_nc(), in_maps, list(range(NCORES)), trace=trace
    )
    return _gather(res.results), res


def _self_check(x: np.ndarray, out: np.ndarray) -> bool:
    """M is a product of orthogonal factors, so ||out_row|| == ||x_row||.

    A cheap reference-free integrity check that catches the rare transient
    corruption seen when an execution races stale device state.  The bf16
    pipeline keeps the max row-norm deviation ~1.1e-3; real corruption is
    orders of magnitude larger.
    """
    xn = np.linalg.norm(np.asarray(x, dtype=np.float64), axis=1)
    on = np.linalg.norm(out.astype(np.float64), axis=1)
    return bool(np.max(np.abs(on - xn) / np.maximum(xn, 1e-6)) < 5e-3)


def kernel(x: np.ndarray, theta: np.ndarray) -> np.ndarray:
    for attempt in range(3):
        out, _ = run(x, theta, trace=False)
        if _self_check(x, out):
            return out
    return out


# revision 22
# speedup vs baseline: 1.1049x; 1.0139x over previous
"""Clements-mesh kernel for Trainium2 (8 NeuronCores, data-parallel).

The reference applies 64 layers of 2x2 Givens-like rotations (alternating
even/odd pair offsets) to x [32768, 256].  Each layer is right-multiplication
by a 256x256 block-diagonal orthogonal matrix U_l, so the whole network is
out = x @ (U_0 @ U_1 @ ... @ U_63) = x @ M with M a dense 256x256 matrix that
only depends on the tiny theta [64, 128].  M is built on host in float64;
the device kernel is a single [4096, 256] @ [256, 256] matmul per core.

Precision: the correctness gate is rel_err < 2e-2, so both x and M are sent
as single bf16 (RTNE) and the result is rounded to bf16 before the output
DMA; accumulation is exact f32 in PSUM.  Measured end-to-end rel err vs the
reference is ~2.9e-3 (7x margin).  This halves HBM traffic vs an x-hi/lo
split with f32 output: 2.2 MiB in + 2.1 MiB out per core, ~12 us at the
~360 GB/s per-core DMA roofline, which is what the kernel is bound by.

Device layout: TensorE contracts over the partition dim of both operands, so
x is shipped feature-major (host pre-transpose) and column-packed in DMA
stream order so every input chunk is ONE contiguous DMA:
  xin [128, 8704] bf16 = [M_kc0 | M_kc1 | X0_kc0 | X0_kc1 | ... | X4_kc1]
where kc = contraction chunk of 128 features and Xi are batch-column chunks
of width CHUNK_W[i].  out^T[j, b] = sum_k M[k, j] x^T[k, b] accumulates over
kc0+kc1 into one PSUM bank per (512-batch block, output-feature half); banks
are drained (with f32->bf16 cast) to SBUF by DVE (jc0) / ACT (jc1) since DMA
cannot read PSUM, then DMAed out feature-major; the host transposes back and
upcasts to f32 while gathering.

Scheduling: hand-built engine programs with explicit semaphores, no Tile
barriers.  The all-engine init barrier + dma_reset of earlier versions
(~3.5 us) is replaced by a semaphore gate: GpSimd clears the data semaphores
then raises start_sem; everything except the first input DMA (receipted on
its own never-start-cleared c0_sem) is gated behind it.  End-of-run GpSimd
clears make the NEFF re-executable; a reference-free row-norm self-check
with retry in kernel() guards the rare stale-device-state corruption.
"""

import sys

import numpy as np

if "/opt/trn_rl_repo" not in sys.path:
    sys.path.insert(0, "/opt/trn_rl_repo")

import concourse.bass as bass
import concourse.mybir as mybir
from concourse.tile import TileContext

D = 256          # feature dim
B = 32768        # batch
NCORES = 8
BS = B // NCORES  # 4096 batch rows per core
P = 128          # SBUF partitions
NB = 512         # batch columns per matmul (one fp32 PSUM bank)
NBLK = BS // NB  # 8 batch blocks
F32 = mybir.dt.float32
BF16 = mybir.dt.bfloat16

# xin column layout: [M_kc0 | M_kc1 | b0_kc0 | b0_kc1 | b1_kc0 | b1_kc1 |
# ... | b7_kc1] — the two 256-col M blocks, then per 512-batch-block pairs
# of contraction halves, in exact PE consumption order.  DMA boundaries
# (below) are chosen so the PE's first matmul depends only on the first
# 256 KB, and later transfers stay ahead of PE consumption (a >100ns PE
# idle gap drops the p-state from 2.4 to 1.2 GHz for ~1-3 us).
XIN_W = 2 * D + 2 * BS  # 8704

# Input DMA column ranges: d0 = M + b0_kc0, d1 = b0_kc1, then one DMA per
# batch block b1..b7 so per-block receipts release the PE as early as
# possible.  d0/d1 are receipted on c0_sem (+16 each), the rest on in_sem
# (+16 each).
IN_DMAS = [
    (0, 1024),
    (1024, 1536),
    (1536, 2560),
    (2560, 3584),
    (3584, 4608),
    (4608, 5632),
    (5632, 6656),
    (6656, 7680),
    (7680, XIN_W),
]
# in_sem threshold (x16) the PE must reach before starting batch block bb
# (b0 is handled specially via c0_sem).
_BB_THR = [0, 1, 2, 3, 4, 5, 6, 7]


def _xcol(bb: int, kc: int) -> int:
    return 2 * D + bb * 2 * NB + kc * NB


_NC_CACHE = {}


def _fused_matrix(theta: np.ndarray) -> np.ndarray:
    """M = U_0 @ U_1 @ ... @ U_63 in float64."""
    theta = np.asarray(theta, dtype=np.float64)
    M = np.eye(D, dtype=np.float64)
    for layer in range(theta.shape[0]):
        th = theta[layer]
        if layer % 2 == 0:
            npairs = D // 2
            i_idx = np.arange(0, D - 1, 2)
        else:
            npairs = D // 2 - 1
            i_idx = np.arange(1, D - 2, 2)
        j_idx = i_idx + 1
        c = np.cos(2.0 * th[:npairs])
        s = np.sin(2.0 * th[:npairs])
        Mi = M[:, i_idx].copy()
        Mj = M[:, j_idx]
        M[:, i_idx] = c * Mi + s * Mj
        M[:, j_idx] = s * Mi - c * Mj
    return M


def _legalize_waits(nc: bass.Bass, max_waits: int = 1) -> None:
    """Split instructions carrying more than ``max_waits`` sync waits.

    This walrus build rejects instructions with multiple sync-wait commands.
    Excess waits move to injected same-engine NoOps immediately before the
    instruction, which is semantically identical: the engine blocks on each
    wait in sequence before executing the original instruction.
    """
    for fn in nc.m.functions:
        for blk in fn.blocks:
            insts = blk.instructions
            i = 0
            while i < len(insts):
                inst = insts[i]
                si = inst.sync_info
                if si is not None and len(si.on_wait) > max_waits:
                    waits = list(si.on_wait)
                    keep, extra = waits[-max_waits:], waits[:-max_waits]
                    for k, w in enumerate(extra):
                        nop = mybir.InstNoOp(
                            name=f"{inst.name}-waitsplit-{k}", ins=[], outs=[]
                        )
                        nop.engine = inst.engine
                        nop.sync_info = mybir.SyncInfo(on_wait=[w], on_update=[])
                        insts.insert(i, nop)
                        i += 1
                    inst.sync_info = mybir.SyncInfo(
                        on_wait=keep, on_update=list(si.on_update)
                    )
                i += 1


def _strip_barriers(nc: bass.Bass) -> None:
    """Remove ALL all-engine EVSEM barrier butterflies + drains.

    Ordering is carried entirely by our semaphore protocol: GpSimd's
    start-of-run semaphore clears gate every semaphore producer via
    start_sem (the one ungated input DMA receipts on c0_sem, which is
    never start-cleared), and GpSimd's end-of-run clears run after the
    final output-DMA write receipt.
    """
    for fn in nc.m.functions:
        for blk in fn.blocks:
            insts = blk.instructions
            keep = [
                i
                for i in insts
                if not (
                    type(i).__name__ == "InstDrain"
                    or (
                        type(i).__name__ == "InstEventSemaphore"
                        and i.name.startswith("barrier")
                    )
                )
            ]
            if len(keep) != len(insts):
                insts[:] = keep


def _build_nc_raw() -> bass.Bass:
    from contextlib import ExitStack

    nc = bass.Bass()
    xin = nc.declare_dram_parameter("xin", [P, XIN_W], BF16, isOutput=False)
    outT = nc.declare_dram_parameter("outT", [2, P, BS], BF16, isOutput=True)

    NWARM = 7       # full-size (512-row) p-state warmup matmuls
    NWARM_FINE = 6  # quarter-size tail warmups for a fine-grained hand-off
    # PSUM banks (per jc) per out-DMA; tapered so the final transfer (which
    # the kernel-end drain effectively waits behind) is a single 128 KB bank.
    OGS = [2, 3, 3]
    assert sum(OGS) == NBLK

    with ExitStack() as ctx:
        x_sb = ctx.enter_context(nc.sbuf_tensor("x_sb", [P, XIN_W], BF16))
        o_sb = ctx.enter_context(nc.sbuf_tensor("o_sb", [P, 2 * BS], BF16))
        ps = [
            ctx.enter_context(nc.psum_tensor(f"ps{b}", [P, NB], F32))
            for b in range(8)
        ]
        c0_sem = ctx.enter_context(nc.semaphore("c0_sem"))
        in_sem = ctx.enter_context(nc.semaphore("in_sem"))
        pe_sem = ctx.enter_context(nc.semaphore("pe_sem"))
        dve_sem = ctx.enter_context(nc.semaphore("dve_sem"))
        act_sem = ctx.enter_context(nc.semaphore("act_sem"))
        out_sem = ctx.enter_context(nc.semaphore("out_sem"))
        start_sem = ctx.enter_context(nc.semaphore("start_sem"))
        block = ctx.enter_context(nc.Block())

        # Group g = 2*bb + jc fills PSUM bank g % 8 with kc0+kc1 accumulated
        # matmuls; jc0 banks drain on DVE, jc1 banks on ACT (f32 -> bf16).

        @block.sync
        def _(sp):
            # The first two DMAs (M + b0_kc0, then b0_kc1) go out
            # immediately, receipted on c0_sem which GpSimd never clears at
            # start-of-run, so the start_sem gate cannot erase their
            # receipts.
            for di, (lo, hi) in enumerate(IN_DMAS[:2]):
                sp.dma_start(out=x_sb[:, lo:hi], in_=xin[:, lo:hi]).then_inc(
                    c0_sem, 16
                )
            # Everything else waits for GpSimd's semaphore clears.
            sp.wait_ge(start_sem, 1)
            for lo, hi in IN_DMAS[2:]:
                sp.dma_start(out=x_sb[:, lo:hi], in_=xin[:, lo:hi]).then_inc(
                    in_sem, 16
                )
            # Output DMAs issued in drain-completion order behind the input
            # stream.  Receipts land on out_sem which nothing waits on
            # (walrus requires a completion semaphore): the SP queue itself
            # retires only after the last pseudo-DMA transfer, and the
            # runtime's end-of-execution teardown quiesces the DMA path
            # before results are read.
            #
            # Crucially, hold output DMAs until the input stream is nearly
            # done: out transfers ride different hardware rings and the DMA
            # engines round-robin across rings, so an early out-DMA steals
            # bandwidth from not-yet-transferred input chunks, starving the
            # PE (and its drains, and thus the tail) for longer than the out
            # transfer gains.  Gating on the SECOND-TO-LAST input receipt
            # splices the first out transfer (issue + descriptor-gen latency
            # ~1.4 us) right behind the last input transfer, keeping the DMA
            # engines saturated to the end.
            sp.wait_ge(in_sem, 16 * (len(IN_DMAS) - 4))
            done = 0
            for og in OGS:
                for jc in range(2):
                    sem = dve_sem if jc == 0 else act_sem
                    sp.wait_ge(sem, done + og)
                    lo, hi = done * NB, (done + og) * NB
                    sp.dma_start(
                        out=outT[jc][:, lo:hi],
                        in_=o_sb[:, jc * BS + lo : jc * BS + hi],
                    ).then_inc(out_sem, 16)
                done += og

        @block.tensor
        def _(pe):
            def mm(bb, jc, kc, start, stop, inc=False):
                m = pe.matmul(
                    ps[(2 * bb + jc) % 8][:],
                    lhsT=x_sb[:, kc * D + jc * P : kc * D + (jc + 1) * P],
                    rhs=x_sb[:, _xcol(bb, kc) : _xcol(bb, kc) + NB],
                    start=start,
                    stop=stop,
                    skip_group_check=True,
                )
                if inc:
                    m.then_inc(pe_sem, 1)

            # Warm the PE p-state on garbage SBUF while the first input DMA
            # lands; bank 7's real group later overwrites this via
            # start=True.  The clocks of the preamble/DMA path and of the
            # warmup matmuls co-vary run to run, so a fixed warmup count
            # tracks the data-arrival time well; the tail of the warmup run
            # uses quarter-size matmuls so the hand-off to the first real
            # matmul is fine-grained (a PE idle gap would reset the p-state
            # ramp and double early matmul latency).
            for _w in range(NWARM):
                pe.matmul(
                    ps[7][:],
                    lhsT=x_sb[:, 0:P],
                    rhs=x_sb[:, 2 * D : 2 * D + NB],
                    start=True,
                    stop=True,
                )
            for _w in range(NWARM_FINE):
                pe.matmul(
                    ps[7][:, 0 : NB // 4],
                    lhsT=x_sb[:, 0:P],
                    rhs=x_sb[:, 2 * D : 2 * D + NB // 4],
                    start=True,
                    stop=True,
                )
            # Never produce a pe_sem increment before GpSimd's clears are
            # done (the c0 DMAs alone could otherwise race them).
            pe.wait_ge(start_sem, 1)
            # Block 0 runs in kc-pair order — both jc matmuls of kc0 first —
            # so work can start before b0_kc1 (second DMA) has landed.
            pe.wait_ge(c0_sem, 16)  # M blocks + b0_kc0
            mm(0, 0, 0, start=True, stop=False)
            mm(0, 1, 0, start=True, stop=False)
            pe.wait_ge(c0_sem, 32)  # b0_kc1
            mm(0, 0, 1, start=False, stop=True, inc=True)
            mm(0, 1, 1, start=False, stop=True, inc=True)
            last_thr = 0
            for bb in range(1, NBLK):
                if _BB_THR[bb] > last_thr:
                    last_thr = _BB_THR[bb]
                    pe.wait_ge(in_sem, 16 * last_thr)
                for jc in range(2):
                    g = 2 * bb + jc
                    if g >= 8:
                        prev = g - 8
                        sem = dve_sem if prev % 2 == 0 else act_sem
                        pe.wait_ge(sem, prev // 2 + 1)
                    mm(bb, jc, 0, start=True, stop=False)
                    mm(bb, jc, 1, start=False, stop=True, inc=True)

        @block.vector
        def _(dve):
            # Tiny delay op: give GpSimd's start-of-run clears time to land
            # before our first wait could observe stale values.
            dve.memset(o_sb[:, 0:8], 0.0)
            for i in range(NBLK):  # jc0 groups: g = 2i
                dve.wait_ge(pe_sem, 2 * i + 1)
                dve.tensor_copy(
                    o_sb[:, i * NB : (i + 1) * NB], ps[(2 * i) % 8][:]
                ).then_inc(dve_sem, 1)

        @block.scalar
        def _(act):
            # Tiny delay op; also triggers the one-time ACT table load well
            # before the first real drain needs it.
            act.copy(o_sb[:, BS : BS + 8], o_sb[:, BS : BS + 8])
            for i in range(NBLK):  # jc1 groups: g = 2i + 1
                act.wait_ge(pe_sem, 2 * i + 2)
                act.copy(
                    o_sb[:, BS + i * NB : BS + (i + 1) * NB], ps[(2 * i + 1) % 8][:]
                ).then_inc(act_sem, 1)

        @block.gpsimd
        def _(gp):
            # Start-of-run: zero the data semaphores, then release everything
            # via start_sem.  (c0_sem deliberately not cleared here: its DMA
            # is dispatched ungated, so a start-clear could erase in-flight
            # receipts.)
            for s in (in_sem, pe_sem, dve_sem, act_sem, out_sem):
                gp.sem_clear(s)
            gp.sem_inc(start_sem, 1)
            # End-of-run: once the last drains are done (i.e. every sem this
            # NEFF waits on has passed its final wait), reset the two
            # semaphores that are NOT start-of-run-cleared so the NEFF is
            # re-executable.  Finishes under the shadow of the final output
            # DMAs still retiring on the SP queue.
            gp.wait_ge(dve_sem, NBLK)
            gp.wait_ge(act_sem, NBLK)
            gp.sem_clear(c0_sem)
            gp.sem_clear(start_sem)

    _strip_barriers(nc)
    _legalize_waits(nc)
    return nc


def _get_nc() -> bass.Bass:
    if "nc" not in _NC_CACHE:
        _NC_CACHE["nc"] = _build_nc_raw()
    return _NC_CACHE["nc"]


def _make_in_maps(x: np.ndarray, theta: np.ndarray):
    import ml_dtypes

    x = np.ascontiguousarray(np.asarray(x), dtype=np.float32)
    mh = _fused_matrix(theta).astype(np.float32).astype(ml_dtypes.bfloat16)

    xr = x.reshape(NCORES, BS, D)
    in_maps = []
    for c in range(NCORES):
        xt = np.ascontiguousarray(xr[c].T).astype(ml_dtypes.bfloat16)
        cols = [mh[:P], mh[P:]]
        for bb in range(NBLK):
            cols.append(xt[:P, bb * NB : (bb + 1) * NB])
            cols.append(xt[P:, bb * NB : (bb + 1) * NB])
        in_maps.append({"xin": np.ascontiguousarray(np.concatenate(cols, axis=1))})
    return in_maps


def _gather(results) -> np.ndarray:
    out = np.empty((B, D), dtype=np.float32)
    for c in range(NCORES):
        oT = np.asarray(results[c]["outT"])  # [2, 128, 4096] bf16
        out[c * BS : (c + 1) * BS, :P] = oT[0].T.astype(np.float32)
        out[c * BS : (c + 1) * BS, P:] = oT[1].T.astype(np.float32)
    return out


def run(x: np.ndarray, theta: np.ndarray, trace: bool = False):
    """Returns (out, BassKernelResults)."""
    from concourse.bass_utils import run_bass_kernel_spmd

    in_maps = _make_in_maps(x, theta)
    res = run_bass_kernel_spmd(
        _get_nc(), in_maps, list(range(NCORES)), trace=trace
    )
    return _gather(res.results), res


def _self_check(x: np.ndarray, out: np.ndarray) -> bool:
    """M is a product of orthogonal factors, so ||out_row|| == ||x_row||.

    A cheap reference-free integrity check that catches the rare transient
    corruption seen when an execution races stale device state.  The bf16
    pipeline keeps the max row-norm deviation ~1.1e-3; real corruption is
    orders of magnitude larger.
    """
    xn = np.linalg.norm(np.asarray(x, dtype=np.float64), axis=1)
    on = np.linalg.norm(out.astype(np.float64), axis=1)
    return bool(np.max(np.abs(on - xn) / np.maximum(xn, 1e-6)) < 5e-3)


def kernel(x: np.ndarray, theta: np.ndarray) -> np.ndarray:
    for attempt in range(3):
        out, _ = run(x, theta, trace=False)
        if _self_check(x, out):
            return out
    return out


# revision 23
# speedup vs baseline: 1.1095x; 1.0041x over previous
"""Clements-mesh kernel for Trainium2 (8 NeuronCores, data-parallel).

The reference applies 64 layers of 2x2 Givens-like rotations (alternating
even/odd pair offsets) to x [32768, 256].  Each layer is right-multiplication
by a 256x256 block-diagonal orthogonal matrix U_l, so the whole network is
out = x @ (U_0 @ U_1 @ ... @ U_63) = x @ M with M a dense 256x256 matrix that
only depends on the tiny theta [64, 128].  M is built on host in float64;
the device kernel is a single [4096, 256] @ [256, 256] matmul per core.

Precision: the correctness gate is rel_err < 2e-2, so both x and M are sent
as single bf16 (RTNE) and the result is rounded to bf16 before the output
DMA; accumulation is exact f32 in PSUM.  Measured end-to-end rel err vs the
reference is ~2.9e-3 (7x margin).  This halves HBM traffic vs an x-hi/lo
split with f32 output: 2.2 MiB in + 2.1 MiB out per core, ~12 us at the
~360 GB/s per-core DMA roofline, which is what the kernel is bound by.

Device layout: TensorE contracts over the partition dim of both operands, so
x is shipped feature-major (host pre-transpose) and column-packed in DMA
stream order so every input chunk is ONE contiguous DMA:
  xin [128, 8704] bf16 = [M_kc0 | M_kc1 | X0_kc0 | X0_kc1 | ... | X4_kc1]
where kc = contraction chunk of 128 features and Xi are batch-column chunks
of width CHUNK_W[i].  out^T[j, b] = sum_k M[k, j] x^T[k, b] accumulates over
kc0+kc1 into one PSUM bank per (512-batch block, output-feature half); banks
are drained (with f32->bf16 cast) to SBUF by DVE (jc0) / ACT (jc1) since DMA
cannot read PSUM, then DMAed out feature-major; the host transposes back and
upcasts to f32 while gathering.

Scheduling: hand-built engine programs with explicit semaphores, no Tile
barriers.  The all-engine init barrier + dma_reset of earlier versions
(~3.5 us) is replaced by a semaphore gate: GpSimd clears the data semaphores
then raises start_sem; everything except the first input DMA (receipted on
its own never-start-cleared c0_sem) is gated behind it.  End-of-run GpSimd
clears make the NEFF re-executable; a reference-free row-norm self-check
with retry in kernel() guards the rare stale-device-state corruption.
"""

import sys

import numpy as np

if "/opt/trn_rl_repo" not in sys.path:
    sys.path.insert(0, "/opt/trn_rl_repo")

import concourse.bass as bass
import concourse.mybir as mybir
from concourse.tile import TileContext

D = 256          # feature dim
B = 32768        # batch
NCORES = 8
BS = B // NCORES  # 4096 batch rows per core
P = 128          # SBUF partitions
NB = 512         # batch columns per matmul (one fp32 PSUM bank)
NBLK = BS // NB  # 8 batch blocks
F32 = mybir.dt.float32
BF16 = mybir.dt.bfloat16

# xin column layout: [M_kc0 | M_kc1 | b0_kc0 | b0_kc1 | b1_kc0 | b1_kc1 |
# ... | b7_kc1] — the two 256-col M blocks, then per 512-batch-block pairs
# of contraction halves, in exact PE consumption order.  DMA boundaries
# (below) are chosen so the PE's first matmul depends only on the first
# 256 KB, and later transfers stay ahead of PE consumption (a >100ns PE
# idle gap drops the p-state from 2.4 to 1.2 GHz for ~1-3 us).
XIN_W = 2 * D + 2 * BS  # 8704

# Input DMA column ranges: d0 = M + b0_kc0, d1 = b0_kc1, then one DMA per
# batch block b1..b7 so per-block receipts release the PE as early as
# possible.  d0/d1 are receipted on c0_sem (+16 each), the rest on in_sem
# (+16 each).
IN_DMAS = [
    (0, 1024),
    (1024, 1536),
    (1536, 2560),
    (2560, 3584),
    (3584, 4608),
    (4608, 5632),
    (5632, 6656),
    (6656, 7680),
    (7680, XIN_W),
]
# in_sem threshold (x16) the PE must reach before starting batch block bb
# (b0 is handled specially via c0_sem).
_BB_THR = [0, 1, 2, 3, 4, 5, 6, 7]


def _xcol(bb: int, kc: int) -> int:
    return 2 * D + bb * 2 * NB + kc * NB


_NC_CACHE = {}


def _fused_matrix(theta: np.ndarray) -> np.ndarray:
    """M = U_0 @ U_1 @ ... @ U_63 in float64."""
    theta = np.asarray(theta, dtype=np.float64)
    M = np.eye(D, dtype=np.float64)
    for layer in range(theta.shape[0]):
        th = theta[layer]
        if layer % 2 == 0:
            npairs = D // 2
            i_idx = np.arange(0, D - 1, 2)
        else:
            npairs = D // 2 - 1
            i_idx = np.arange(1, D - 2, 2)
        j_idx = i_idx + 1
        c = np.cos(2.0 * th[:npairs])
        s = np.sin(2.0 * th[:npairs])
        Mi = M[:, i_idx].copy()
        Mj = M[:, j_idx]
        M[:, i_idx] = c * Mi + s * Mj
        M[:, j_idx] = s * Mi - c * Mj
    return M


def _legalize_waits(nc: bass.Bass, max_waits: int = 1) -> None:
    """Split instructions carrying more than ``max_waits`` sync waits.

    This walrus build rejects instructions with multiple sync-wait commands.
    Excess waits move to injected same-engine NoOps immediately before the
    instruction, which is semantically identical: the engine blocks on each
    wait in sequence before executing the original instruction.
    """
    for fn in nc.m.functions:
        for blk in fn.blocks:
            insts = blk.instructions
            i = 0
            while i < len(insts):
                inst = insts[i]
                si = inst.sync_info
                if si is not None and len(si.on_wait) > max_waits:
                    waits = list(si.on_wait)
                    keep, extra = waits[-max_waits:], waits[:-max_waits]
                    for k, w in enumerate(extra):
                        nop = mybir.InstNoOp(
                            name=f"{inst.name}-waitsplit-{k}", ins=[], outs=[]
                        )
                        nop.engine = inst.engine
                        nop.sync_info = mybir.SyncInfo(on_wait=[w], on_update=[])
                        insts.insert(i, nop)
                        i += 1
                    inst.sync_info = mybir.SyncInfo(
                        on_wait=keep, on_update=list(si.on_update)
                    )
                i += 1


def _strip_barriers(nc: bass.Bass) -> None:
    """Remove ALL all-engine EVSEM barrier butterflies + drains.

    Ordering is carried entirely by our semaphore protocol: GpSimd's
    start-of-run semaphore clears gate every semaphore producer via
    start_sem (the one ungated input DMA receipts on c0_sem, which is
    never start-cleared), and GpSimd's end-of-run clears run after the
    final output-DMA write receipt.
    """
    for fn in nc.m.functions:
        for blk in fn.blocks:
            insts = blk.instructions
            keep = [
                i
                for i in insts
                if not (
                    type(i).__name__ == "InstDrain"
                    or (
                        type(i).__name__ == "InstEventSemaphore"
                        and i.name.startswith("barrier")
                    )
                )
            ]
            if len(keep) != len(insts):
                insts[:] = keep


def _build_nc_raw() -> bass.Bass:
    from contextlib import ExitStack

    nc = bass.Bass()
    xin = nc.declare_dram_parameter("xin", [P, XIN_W], BF16, isOutput=False)
    outT = nc.declare_dram_parameter("outT", [2, P, BS], BF16, isOutput=True)

    NWARM = 7       # full-size (512-row) p-state warmup matmuls
    NWARM_FINE = 6  # quarter-size tail warmups for a fine-grained hand-off
    # PSUM banks (per jc) per out-DMA; tapered so the final transfer (which
    # the kernel-end drain effectively waits behind) is a single 128 KB bank.
    OGS = [2, 3, 3]
    assert sum(OGS) == NBLK

    with ExitStack() as ctx:
        x_sb = ctx.enter_context(nc.sbuf_tensor("x_sb", [P, XIN_W], BF16))
        o_sb = ctx.enter_context(nc.sbuf_tensor("o_sb", [P, 2 * BS], BF16))
        ps = [
            ctx.enter_context(nc.psum_tensor(f"ps{b}", [P, NB], F32))
            for b in range(8)
        ]
        c0_sem = ctx.enter_context(nc.semaphore("c0_sem"))
        in_sem = ctx.enter_context(nc.semaphore("in_sem"))
        pe_sem = ctx.enter_context(nc.semaphore("pe_sem"))
        dve_sem = ctx.enter_context(nc.semaphore("dve_sem"))
        act_sem = ctx.enter_context(nc.semaphore("act_sem"))
        out_sem = ctx.enter_context(nc.semaphore("out_sem"))
        start_sem = ctx.enter_context(nc.semaphore("start_sem"))
        block = ctx.enter_context(nc.Block())

        # Group g = 2*bb + jc fills PSUM bank g % 8 with kc0+kc1 accumulated
        # matmuls; jc0 banks drain on DVE, jc1 banks on ACT (f32 -> bf16).

        @block.sync
        def _(sp):
            # The first two DMAs (M + b0_kc0, then b0_kc1) go out
            # immediately, receipted on c0_sem which GpSimd never clears at
            # start-of-run, so the start_sem gate cannot erase their
            # receipts.
            for di, (lo, hi) in enumerate(IN_DMAS[:2]):
                sp.dma_start(out=x_sb[:, lo:hi], in_=xin[:, lo:hi]).then_inc(
                    c0_sem, 16
                )
            # Everything else waits for GpSimd's semaphore clears.
            sp.wait_ge(start_sem, 1)
            for lo, hi in IN_DMAS[2:]:
                sp.dma_start(out=x_sb[:, lo:hi], in_=xin[:, lo:hi]).then_inc(
                    in_sem, 16
                )
            # Output DMAs issued in drain-completion order behind the input
            # stream.  Receipts land on out_sem which nothing waits on
            # (walrus requires a completion semaphore): the SP queue itself
            # retires only after the last pseudo-DMA transfer, and the
            # runtime's end-of-execution teardown quiesces the DMA path
            # before results are read.
            #
            # Crucially, hold output DMAs until the input stream is nearly
            # done: out transfers ride different hardware rings and the DMA
            # engines round-robin across rings, so an early out-DMA steals
            # bandwidth from not-yet-transferred input chunks, starving the
            # PE (and its drains, and thus the tail) for longer than the out
            # transfer gains.  Gating on the SECOND-TO-LAST input receipt
            # splices the first out transfer (issue + descriptor-gen latency
            # ~1.4 us) right behind the last input transfer, keeping the DMA
            # engines saturated to the end.
            sp.wait_ge(in_sem, 16 * (len(IN_DMAS) - 5))
            done = 0
            for og in OGS:
                for jc in range(2):
                    sem = dve_sem if jc == 0 else act_sem
                    sp.wait_ge(sem, done + og)
                    lo, hi = done * NB, (done + og) * NB
                    sp.dma_start(
                        out=outT[jc][:, lo:hi],
                        in_=o_sb[:, jc * BS + lo : jc * BS + hi],
                    ).then_inc(out_sem, 16)
                done += og

        @block.tensor
        def _(pe):
            def mm(bb, jc, kc, start, stop, inc=False):
                m = pe.matmul(
                    ps[(2 * bb + jc) % 8][:],
                    lhsT=x_sb[:, kc * D + jc * P : kc * D + (jc + 1) * P],
                    rhs=x_sb[:, _xcol(bb, kc) : _xcol(bb, kc) + NB],
                    start=start,
                    stop=stop,
                    skip_group_check=True,
                )
                if inc:
                    m.then_inc(pe_sem, 1)

            # Warm the PE p-state on garbage SBUF while the first input DMA
            # lands; bank 7's real group later overwrites this via
            # start=True.  The clocks of the preamble/DMA path and of the
            # warmup matmuls co-vary run to run, so a fixed warmup count
            # tracks the data-arrival time well; the tail of the warmup run
            # uses quarter-size matmuls so the hand-off to the first real
            # matmul is fine-grained (a PE idle gap would reset the p-state
            # ramp and double early matmul latency).
            for _w in range(NWARM):
                pe.matmul(
                    ps[7][:],
                    lhsT=x_sb[:, 0:P],
                    rhs=x_sb[:, 2 * D : 2 * D + NB],
                    start=True,
                    stop=True,
                )
            for _w in range(NWARM_FINE):
                pe.matmul(
                    ps[7][:, 0 : NB // 4],
                    lhsT=x_sb[:, 0:P],
                    rhs=x_sb[:, 2 * D : 2 * D + NB // 4],
                    start=True,
                    stop=True,
                )
            # Never produce a pe_sem increment before GpSimd's clears are
            # done (the c0 DMAs alone could otherwise race them).
            pe.wait_ge(start_sem, 1)
            # Block 0 runs in kc-pair order — both jc matmuls of kc0 first —
            # so work can start before b0_kc1 (second DMA) has landed.
            pe.wait_ge(c0_sem, 16)  # M blocks + b0_kc0
            mm(0, 0, 0, start=True, stop=False)
            mm(0, 1, 0, start=True, stop=False)
            pe.wait_ge(c0_sem, 32)  # b0_kc1
            mm(0, 0, 1, start=False, stop=True, inc=True)
            mm(0, 1, 1, start=False, stop=True, inc=True)
            last_thr = 0
            for bb in range(1, NBLK):
                if _BB_THR[bb] > last_thr:
                    last_thr = _BB_THR[bb]
                    pe.wait_ge(in_sem, 16 * last_thr)
                for jc in range(2):
                    g = 2 * bb + jc
                    if g >= 8:
                        prev = g - 8
                        sem = dve_sem if prev % 2 == 0 else act_sem
                        pe.wait_ge(sem, prev // 2 + 1)
                    mm(bb, jc, 0, start=True, stop=False)
                    mm(bb, jc, 1, start=False, stop=True, inc=True)

        @block.vector
        def _(dve):
            # Tiny delay op: give GpSimd's start-of-run clears time to land
            # before our first wait could observe stale values.
            dve.memset(o_sb[:, 0:8], 0.0)
            for i in range(NBLK):  # jc0 groups: g = 2i
                dve.wait_ge(pe_sem, 2 * i + 1)
                dve.tensor_copy(
                    o_sb[:, i * NB : (i + 1) * NB], ps[(2 * i) % 8][:]
                ).then_inc(dve_sem, 1)

        @block.scalar
        def _(act):
            # Tiny delay op; also triggers the one-time ACT table load well
            # before the first real drain needs it.
            act.copy(o_sb[:, BS : BS + 8], o_sb[:, BS : BS + 8])
            for i in range(NBLK):  # jc1 groups: g = 2i + 1
                act.wait_ge(pe_sem, 2 * i + 2)
                act.copy(
                    o_sb[:, BS + i * NB : BS + (i + 1) * NB], ps[(2 * i + 1) % 8][:]
                ).then_inc(act_sem, 1)

        @block.gpsimd
        def _(gp):
            # Start-of-run: zero the data semaphores, then release everything
            # via start_sem.  (c0_sem deliberately not cleared here: its DMA
            # is dispatched ungated, so a start-clear could erase in-flight
            # receipts.)
            for s in (in_sem, pe_sem, dve_sem, act_sem, out_sem):
                gp.sem_clear(s)
            gp.sem_inc(start_sem, 1)
            # End-of-run: once the last drains are done (i.e. every sem this
            # NEFF waits on has passed its final wait), reset the two
            # semaphores that are NOT start-of-run-cleared so the NEFF is
            # re-executable.  Finishes under the shadow of the final output
            # DMAs still retiring on the SP queue.
            gp.wait_ge(dve_sem, NBLK)
            gp.wait_ge(act_sem, NBLK)
            gp.sem_clear(c0_sem)
            gp.sem_clear(start_sem)

    _strip_barriers(nc)
    _legalize_waits(nc)
    return nc


def _get_nc() -> bass.Bass:
    if "nc" not in _NC_CACHE:
        _NC_CACHE["nc"] = _build_nc_raw()
    return _NC_CACHE["nc"]


def _make_in_maps(x: np.ndarray, theta: np.ndarray):
    import ml_dtypes

    x = np.ascontiguousarray(np.asarray(x), dtype=np.float32)
    mh = _fused_matrix(theta).astype(np.float32).astype(ml_dtypes.bfloat16)

    xr = x.reshape(NCORES, BS, D)
    in_maps = []
    for c in range(NCORES):
        xt = np.ascontiguousarray(xr[c].T).astype(ml_dtypes.bfloat16)
        cols = [mh[:P], mh[P:]]
        for bb in range(NBLK):
            cols.append(xt[:P, bb * NB : (bb + 1) * NB])
            cols.append(xt[P:, bb * NB : (bb + 1) * NB])
        in_maps.append({"xin": np.ascontiguousarray(np.concatenate(cols, axis=1))})
    return in_maps


def _gather(results) -> np.ndarray:
    out = np.empty((B, D), dtype=np.float32)
    for c in range(NCORES):
        oT = np.asarray(results[c]["outT"])  # [2, 128, 4096] bf16
        out[c * BS : (c + 1) * BS, :P] = oT[0].T.astype(np.float32)
        out[c * BS : (c + 1) * BS, P:] = oT[1].T.astype(np.float32)
    return out


def run(x: np.ndarray, theta: np.ndarray, trace: bool = False):
    """Returns (out, BassKernelResults)."""
    from concourse.bass_utils import run_bass_kernel_spmd

    in_maps = _make_in_maps(x, theta)
    res = run_bass_kernel_spmd(
        _get_nc(), in_maps, list(range(NCORES)), trace=trace
    )
    return _gather(res.results), res


def _self_check(x: np.ndarray, out: np.ndarray) -> bool:
    """M is a product of orthogonal factors, so ||out_row|| == ||x_row||.

    A cheap reference-free integrity check that catches the rare transient
    corruption seen when an execution races stale device state.  The bf16
    pipeline keeps the max row-norm deviation ~1.1e-3; real corruption is
    orders of magnitude larger.
    """
    xn = np.linalg.norm(np.asarray(x, dtype=np.float64), axis=1)
    on = np.linalg.norm(out.astype(np.float64), axis=1)
    return bool(np.max(np.abs(on - xn) / np.maximum(xn, 1e-6)) < 5e-3)


def kernel(x: np.ndarray, theta: np.ndarray) -> np.ndarray:
    for attempt in range(3):
        out, _ = run(x, theta, trace=False)
        if _self_check(x, out):
            return out
    return out


# revision 24
# speedup vs baseline: 1.1216x; 1.0109x over previous
"""Clements-mesh kernel for Trainium2 (8 NeuronCores, data-parallel).

The reference applies 64 layers of 2x2 Givens-like rotations (alternating
even/odd pair offsets) to x [32768, 256].  Each layer is right-multiplication
by a 256x256 block-diagonal orthogonal matrix U_l, so the whole network is
out = x @ (U_0 @ U_1 @ ... @ U_63) = x @ M with M a dense 256x256 matrix that
only depends on the tiny theta [64, 128].  M is built on host in float64;
the device kernel is a single [4096, 256] @ [256, 256] matmul per core.

Precision: the correctness gate is rel_err < 2e-2, so both x and M are sent
as single bf16 (RTNE) and the result is rounded to bf16 before the output
DMA; accumulation is exact f32 in PSUM.  Measured end-to-end rel err vs the
reference is ~2.9e-3 (7x margin).  This halves HBM traffic vs an x-hi/lo
split with f32 output: 2.2 MiB in + 2.1 MiB out per core, ~12 us at the
~360 GB/s per-core DMA roofline, which is what the kernel is bound by.

Device layout: TensorE contracts over the partition dim of both operands, so
x is shipped feature-major (host pre-transpose) and column-packed in exact
PE consumption order so every input DMA is one contiguous copy:
  xin [128, 8704] bf16 = [M_kc0 | M_kc1 | b0_kc0 | b0_kc1 | ... | b7_kc1]
where kc = contraction chunk of 128 features and bi are 512-wide batch
blocks.  out^T[j, b] = sum_k M[k, j] x^T[k, b] accumulates over kc0+kc1
into one PSUM bank per (batch block, output-feature half); banks are
drained (with f32->bf16 cast) to SBUF by DVE (jc0) / ACT (jc1) since DMA
cannot read PSUM, then DMAed out feature-major; the host transposes back
and upcasts to f32 while gathering.

The kernel is DMA-bound end to end: the 16 per-core DMA engines stay
saturated from the first input transfer to the last output transfer.  The
schedule exists to keep them (and the PE p-state) from ever going idle:
fine-grained warmup hand-off, per-block input receipts, and output DMAs
issued just early enough that their first transfer splices directly behind
the input stream without stealing bandwidth from it.

Scheduling: hand-built engine programs with explicit semaphores, no Tile
barriers.  The all-engine init barrier + dma_reset of earlier versions
(~3.5 us) is replaced by a semaphore gate: GpSimd clears the data semaphores
then raises start_sem; everything except the first input DMA (receipted on
its own never-start-cleared c0_sem) is gated behind it.  End-of-run GpSimd
clears make the NEFF re-executable; a reference-free row-norm self-check
with retry in kernel() guards the rare stale-device-state corruption.
"""

import sys

import numpy as np

if "/opt/trn_rl_repo" not in sys.path:
    sys.path.insert(0, "/opt/trn_rl_repo")

import concourse.bass as bass
import concourse.mybir as mybir
from concourse.tile import TileContext

D = 256          # feature dim
B = 32768        # batch
NCORES = 8
BS = B // NCORES  # 4096 batch rows per core
P = 128          # SBUF partitions
NB = 512         # batch columns per matmul (one fp32 PSUM bank)
NBLK = BS // NB  # 8 batch blocks
F32 = mybir.dt.float32
BF16 = mybir.dt.bfloat16

# xin column layout: [M_kc0 | M_kc1 | b0_kc0 | b0_kc1 | b1_kc0 | b1_kc1 |
# ... | b7_kc1] — the two 256-col M blocks, then per 512-batch-block pairs
# of contraction halves, in exact PE consumption order.  DMA boundaries
# (below) are chosen so the PE's first matmul depends only on the first
# 256 KB, and later transfers stay ahead of PE consumption (a >100ns PE
# idle gap drops the p-state from 2.4 to 1.2 GHz for ~1-3 us).
XIN_W = 2 * D + 2 * BS  # 8704

# Input DMA column ranges: d0 = M + b0_kc0, d1 = b0_kc1, then one DMA per
# batch block b1..b7 so per-block receipts release the PE as early as
# possible.  d0/d1 are receipted on c0_sem (+16 each), the rest on in_sem
# (+16 each).
IN_DMAS = [
    (0, 1024),
    (1024, 1536),
    (1536, 2560),
    (2560, 3584),
    (3584, 4608),
    (4608, 5632),
    (5632, 6656),
    (6656, 7680),
    (7680, XIN_W),
]
# in_sem threshold (x16) the PE must reach before starting batch block bb
# (b0 is handled specially via c0_sem).
_BB_THR = [0, 1, 2, 3, 4, 5, 6, 7]


def _xcol(bb: int, kc: int) -> int:
    return 2 * D + bb * 2 * NB + kc * NB


_NC_CACHE = {}


def _fused_matrix(theta: np.ndarray) -> np.ndarray:
    """M = U_0 @ U_1 @ ... @ U_63 in float64."""
    theta = np.asarray(theta, dtype=np.float64)
    M = np.eye(D, dtype=np.float64)
    for layer in range(theta.shape[0]):
        th = theta[layer]
        if layer % 2 == 0:
            npairs = D // 2
            i_idx = np.arange(0, D - 1, 2)
        else:
            npairs = D // 2 - 1
            i_idx = np.arange(1, D - 2, 2)
        j_idx = i_idx + 1
        c = np.cos(2.0 * th[:npairs])
        s = np.sin(2.0 * th[:npairs])
        Mi = M[:, i_idx].copy()
        Mj = M[:, j_idx]
        M[:, i_idx] = c * Mi + s * Mj
        M[:, j_idx] = s * Mi - c * Mj
    return M


def _legalize_waits(nc: bass.Bass, max_waits: int = 1) -> None:
    """Split instructions carrying more than ``max_waits`` sync waits.

    This walrus build rejects instructions with multiple sync-wait commands.
    Excess waits move to injected same-engine NoOps immediately before the
    instruction, which is semantically identical: the engine blocks on each
    wait in sequence before executing the original instruction.
    """
    for fn in nc.m.functions:
        for blk in fn.blocks:
            insts = blk.instructions
            i = 0
            while i < len(insts):
                inst = insts[i]
                si = inst.sync_info
                if si is not None and len(si.on_wait) > max_waits:
                    waits = list(si.on_wait)
                    keep, extra = waits[-max_waits:], waits[:-max_waits]
                    for k, w in enumerate(extra):
                        nop = mybir.InstNoOp(
                            name=f"{inst.name}-waitsplit-{k}", ins=[], outs=[]
                        )
                        nop.engine = inst.engine
                        nop.sync_info = mybir.SyncInfo(on_wait=[w], on_update=[])
                        insts.insert(i, nop)
                        i += 1
                    inst.sync_info = mybir.SyncInfo(
                        on_wait=keep, on_update=list(si.on_update)
                    )
                i += 1


def _strip_barriers(nc: bass.Bass) -> None:
    """Remove ALL all-engine EVSEM barrier butterflies + drains.

    Ordering is carried entirely by our semaphore protocol: GpSimd's
    start-of-run semaphore clears gate every semaphore producer via
    start_sem (the one ungated input DMA receipts on c0_sem, which is
    never start-cleared), and GpSimd's end-of-run clears run after the
    final output-DMA write receipt.
    """
    for fn in nc.m.functions:
        for blk in fn.blocks:
            insts = blk.instructions
            keep = [
                i
                for i in insts
                if not (
                    type(i).__name__ == "InstDrain"
                    or (
                        type(i).__name__ == "InstEventSemaphore"
                        and i.name.startswith("barrier")
                    )
                )
            ]
            if len(keep) != len(insts):
                insts[:] = keep


def _build_nc_raw() -> bass.Bass:
    from contextlib import ExitStack

    nc = bass.Bass()
    xin = nc.declare_dram_parameter("xin", [P, XIN_W], BF16, isOutput=False)
    outT = nc.declare_dram_parameter("outT", [2, P, BS], BF16, isOutput=True)

    NWARM = 7       # full-size (512-row) p-state warmup matmuls
    NWARM_FINE = 6  # quarter-size tail warmups for a fine-grained hand-off
    # PSUM banks (per jc) per out-DMA; tapered so the final transfer (which
    # the kernel-end drain effectively waits behind) is a single 128 KB bank.
    OGS = [2, 3, 3]
    assert sum(OGS) == NBLK

    with ExitStack() as ctx:
        x_sb = ctx.enter_context(nc.sbuf_tensor("x_sb", [P, XIN_W], BF16))
        o_sb = ctx.enter_context(nc.sbuf_tensor("o_sb", [P, 2 * BS], BF16))
        ps = [
            ctx.enter_context(nc.psum_tensor(f"ps{b}", [P, NB], F32))
            for b in range(8)
        ]
        c0_sem = ctx.enter_context(nc.semaphore("c0_sem"))
        in_sem = ctx.enter_context(nc.semaphore("in_sem"))
        pe_sem = ctx.enter_context(nc.semaphore("pe_sem"))
        dve_sem = ctx.enter_context(nc.semaphore("dve_sem"))
        act_sem = ctx.enter_context(nc.semaphore("act_sem"))
        out_sem = ctx.enter_context(nc.semaphore("out_sem"))
        start_sem = ctx.enter_context(nc.semaphore("start_sem"))
        block = ctx.enter_context(nc.Block())

        # Group g = 2*bb + jc fills PSUM bank g % 8 with kc0+kc1 accumulated
        # matmuls; jc0 banks drain on DVE, jc1 banks on ACT (f32 -> bf16).

        @block.sync
        def _(sp):
            # The first two DMAs (M + b0_kc0, then b0_kc1) go out
            # immediately, receipted on c0_sem which GpSimd never clears at
            # start-of-run, so the start_sem gate cannot erase their
            # receipts.
            for di, (lo, hi) in enumerate(IN_DMAS[:2]):
                sp.dma_start(out=x_sb[:, lo:hi], in_=xin[:, lo:hi]).then_inc(
                    c0_sem, 16
                )
            # Everything else waits for GpSimd's semaphore clears.
            sp.wait_ge(start_sem, 1)
            for lo, hi in IN_DMAS[2:]:
                sp.dma_start(out=x_sb[:, lo:hi], in_=xin[:, lo:hi]).then_inc(
                    in_sem, 16
                )
            # Output DMAs issued in drain-completion order behind the input
            # stream.  Receipts land on out_sem which nothing waits on
            # (walrus requires a completion semaphore): the SP queue itself
            # retires only after the last pseudo-DMA transfer, and the
            # runtime's end-of-execution teardown quiesces the DMA path
            # before results are read.
            #
            # Crucially, hold output DMAs until the input stream is nearly
            # done: out transfers ride different hardware rings and the DMA
            # engines round-robin across rings, so an early out-DMA steals
            # bandwidth from not-yet-transferred input chunks, starving the
            # PE (and its drains, and thus the tail) for longer than the out
            # transfer gains.  Gating on the SECOND-TO-LAST input receipt
            # splices the first out transfer (issue + descriptor-gen latency
            # ~1.4 us) right behind the last input transfer, keeping the DMA
            # engines saturated to the end.
            sp.wait_ge(in_sem, 16 * (len(IN_DMAS) - 5))
            done = 0
            for og in OGS:
                for jc in range(2):
                    sem = dve_sem if jc == 0 else act_sem
                    sp.wait_ge(sem, done + og)
                    lo, hi = done * NB, (done + og) * NB
                    sp.dma_start(
                        out=outT[jc][:, lo:hi],
                        in_=o_sb[:, jc * BS + lo : jc * BS + hi],
                    ).then_inc(out_sem, 16)
                done += og

        @block.tensor
        def _(pe):
            def mm(bb, jc, kc, start, stop, inc=False):
                m = pe.matmul(
                    ps[(2 * bb + jc) % 8][:],
                    lhsT=x_sb[:, kc * D + jc * P : kc * D + (jc + 1) * P],
                    rhs=x_sb[:, _xcol(bb, kc) : _xcol(bb, kc) + NB],
                    start=start,
                    stop=stop,
                    skip_group_check=True,
                )
                if inc:
                    m.then_inc(pe_sem, 1)

            # Warm the PE p-state on garbage SBUF while the first input DMA
            # lands; bank 7's real group later overwrites this via
            # start=True.  The clocks of the preamble/DMA path and of the
            # warmup matmuls co-vary run to run, so a fixed warmup count
            # tracks the data-arrival time well; the tail of the warmup run
            # uses quarter-size matmuls so the hand-off to the first real
            # matmul is fine-grained (a PE idle gap would reset the p-state
            # ramp and double early matmul latency).
            for _w in range(NWARM):
                pe.matmul(
                    ps[7][:],
                    lhsT=x_sb[:, 0:P],
                    rhs=x_sb[:, 2 * D : 2 * D + NB],
                    start=True,
                    stop=True,
                )
            for _w in range(NWARM_FINE):
                pe.matmul(
                    ps[7][:, 0 : NB // 4],
                    lhsT=x_sb[:, 0:P],
                    rhs=x_sb[:, 2 * D : 2 * D + NB // 4],
                    start=True,
                    stop=True,
                )
            # Never produce a pe_sem increment before GpSimd's clears are
            # done (the c0 DMAs alone could otherwise race them).
            pe.wait_ge(start_sem, 1)
            # Block 0 runs in kc-pair order — both jc matmuls of kc0 first —
            # so work can start before b0_kc1 (second DMA) has landed.
            pe.wait_ge(c0_sem, 16)  # M blocks + b0_kc0
            mm(0, 0, 0, start=True, stop=False)
            mm(0, 1, 0, start=True, stop=False)
            pe.wait_ge(c0_sem, 32)  # b0_kc1
            mm(0, 0, 1, start=False, stop=True, inc=True)
            mm(0, 1, 1, start=False, stop=True, inc=True)
            last_thr = 0
            for bb in range(1, NBLK):
                if _BB_THR[bb] > last_thr:
                    last_thr = _BB_THR[bb]
                    pe.wait_ge(in_sem, 16 * last_thr)
                for jc in range(2):
                    g = 2 * bb + jc
                    if g >= 8:
                        prev = g - 8
                        sem = dve_sem if prev % 2 == 0 else act_sem
                        pe.wait_ge(sem, prev // 2 + 1)
                    mm(bb, jc, 0, start=True, stop=False)
                    mm(bb, jc, 1, start=False, stop=True, inc=True)

        @block.vector
        def _(dve):
            # Tiny delay op: give GpSimd's start-of-run clears time to land
            # before our first wait could observe stale values.
            dve.memset(o_sb[:, 0:8], 0.0)
            for i in range(NBLK):  # jc0 groups: g = 2i
                dve.wait_ge(pe_sem, 2 * i + 1)
                dve.tensor_copy(
                    o_sb[:, i * NB : (i + 1) * NB], ps[(2 * i) % 8][:]
                ).then_inc(dve_sem, 1)

        @block.scalar
        def _(act):
            # Tiny delay op; also triggers the one-time ACT table load well
            # before the first real drain needs it.
            act.copy(o_sb[:, BS : BS + 8], o_sb[:, BS : BS + 8])
            for i in range(NBLK):  # jc1 groups: g = 2i + 1
                act.wait_ge(pe_sem, 2 * i + 2)
                act.copy(
                    o_sb[:, BS + i * NB : BS + (i + 1) * NB], ps[(2 * i + 1) % 8][:]
                ).then_inc(act_sem, 1)

        @block.gpsimd
        def _(gp):
            # Start-of-run: zero the data semaphores, then release everything
            # via start_sem.  (c0_sem deliberately not cleared here: its DMA
            # is dispatched ungated, so a start-clear could erase in-flight
            # receipts.)
            for s in (in_sem, pe_sem, dve_sem, act_sem, out_sem):
                gp.sem_clear(s)
            gp.sem_inc(start_sem, 1)
            # End-of-run: once the last drains are done (i.e. every sem this
            # NEFF waits on has passed its final wait), reset the two
            # semaphores that are NOT start-of-run-cleared so the NEFF is
            # re-executable.  Finishes under the shadow of the final output
            # DMAs still retiring on the SP queue.
            gp.wait_ge(dve_sem, NBLK)
            gp.wait_ge(act_sem, NBLK)
            gp.sem_clear(c0_sem)
            gp.sem_clear(start_sem)

    _strip_barriers(nc)
    _legalize_waits(nc)
    return nc


def _get_nc() -> bass.Bass:
    if "nc" not in _NC_CACHE:
        _NC_CACHE["nc"] = _build_nc_raw()
    return _NC_CACHE["nc"]


def _make_in_maps(x: np.ndarray, theta: np.ndarray):
    import ml_dtypes

    x = np.ascontiguousarray(np.asarray(x), dtype=np.float32)
    mh = _fused_matrix(theta).astype(np.float32).astype(ml_dtypes.bfloat16)

    xr = x.reshape(NCORES, BS, D)
    in_maps = []
    for c in range(NCORES):
        xt = np.ascontiguousarray(xr[c].T).astype(ml_dtypes.bfloat16)
        cols = [mh[:P], mh[P:]]
        for bb in range(NBLK):
            cols.append(xt[:P, bb * NB : (bb + 1) * NB])
            cols.append(xt[P:, bb * NB : (bb + 1) * NB])
        in_maps.append({"xin": np.ascontiguousarray(np.concatenate(cols, axis=1))})
    return in_maps


def _gather(results) -> np.ndarray:
    out = np.empty((B, D), dtype=np.float32)
    for c in range(NCORES):
        oT = np.asarray(results[c]["outT"])  # [2, 128, 4096] bf16
        out[c * BS : (c + 1) * BS, :P] = oT[0].T.astype(np.float32)
        out[c * BS : (c + 1) * BS, P:] = oT[1].T.astype(np.float32)
    return out


def run(x: np.ndarray, theta: np.ndarray, trace: bool = False):
    """Returns (out, BassKernelResults)."""
    from concourse.bass_utils import run_bass_kernel_spmd

    in_maps = _make_in_maps(x, theta)
    res = run_bass_kernel_spmd(
        _get_nc(), in_maps, list(range(NCORES)), trace=trace
    )
    return _gather(res.results), res


def _self_check(x: np.ndarray, out: np.ndarray) -> bool:
    """M is a product of orthogonal factors, so ||out_row|| == ||x_row||.

    A cheap reference-free integrity check that catches the rare transient
    corruption seen when an execution races stale device state.  The bf16
    pipeline keeps the max row-norm deviation ~1.1e-3; real corruption is
    orders of magnitude larger.
    """
    xn = np.linalg.norm(np.asarray(x, dtype=np.float64), axis=1)
    on = np.linalg.norm(out.astype(np.float64), axis=1)
    return bool(np.max(np.abs(on - xn) / np.maximum(xn, 1e-6)) < 5e-3)


def kernel(x: np.ndarray, theta: np.ndarray) -> np.ndarray:
    for attempt in range(3):
        out, _ = run(x, theta, trace=False)
        if _self_check(x, out):
            return out
    return out
